# revision 37
# baseline (speedup 1.0000x reference)
"""Multi-head attention (B=2, N=4096, C=512, H=8) on 8 TRN2 NeuronCores.

Sharding: core c handles batch c//4 and heads {2*(c%4), 2*(c%4)+1}
(data parallel over batch, tensor parallel over heads). Each core
computes its 2 heads' attention plus a partial output projection;
the host sums the 4 partials per batch and adds the bias terms
(b_out and b_v @ W_out, which commutes past softmax-weighted sums).

Compute layout per core (matmul operands bf16, accumulation f32):
  xT    = x.T via 2-byte DMA transpose (x pre-cast bf16 on host)
  qT,kT = per-head rows of (x @ Wq + bq).T etc.    [64, 2, N] bf16
  v     = x @ Wv, DMA-transposed to natural layout with a ones column
          per head (ones turns P@[V|1] into [P@V | rowsum(P)])
  per (512-query block, head), groups of 2 key-chunks:
     scoresT = kT_kb.T(stationary) @ qT   -> PSUM [128, 2, 512] f32
     expT    = Exp(scoresT / sqrt(C))     -> SBUF bf16 (one wide ACT op)
     accum P.T @ [v|1] over kb            -> PSUM [65, 512] f32
  tail (deferred one unit so the PE stream never blocks): reciprocal of
  row 64, partition-broadcast via a DRAM round-trip, normalize+cast bf16
  proj (deferred two units): oT.T @ W_out rows (K=64 per head, accum)
"""

import numpy as np

import concourse.bass as bass
import concourse.mybir as mybir
import concourse.tile as tile
from concourse.bass_utils import run_bass_kernel_spmd
from concourse.vector_clock import ScopedClock

F32 = mybir.dt.float32
BF16 = mybir.dt.bfloat16
AF = mybir.ActivationFunctionType

B, N, C, H = 2, 4096, 512, 8
HD = C // H          # 64
HPC = H // 4         # 2 heads per core
NCORES = 8
NT = N // 128        # 32 key chunks
NCJ = C // 128       # 4 contraction chunks
QB = N // 512        # 8 query blocks
GP = 2               # key chunks per exp group (PSUM banks per scores tile)
NG = NT // GP
NUNITS = QB * HPC
SCALE = 1.0 / float(np.sqrt(C))


def _patch_tail_drain():
    """This walrus build caps sync waits at 1 per non-EventSemaphore
    instruction (2 for EventSemaphore); the stock TileContext tail-drain
    attaches every outstanding wait to one Drain, and the scheduler can
    leave >1 wait on regular instructions. Spill extras onto fresh
    same-engine nops inserted just before the over-subscribed one."""
    if getattr(tile.TileContext, "_drain_patched", False):
        return

    def _spill_excess_waits(nc):
        for fn in nc.m.functions:
            for bb in fn.blocks:
                insts = bb.instructions
                i = 0
                while i < len(insts):
                    inst = insts[i]
                    si = inst.sync_info
                    cap = 2 if isinstance(inst, mybir.InstEventSemaphore) else 1
                    if si is None or len(si.on_wait) <= cap:
                        i += 1
                        continue
                    extra = list(si.on_wait[cap:])
                    si.on_wait[:] = si.on_wait[:cap]
                    for w in extra:
                        nop = nc.engines[inst.engine].nop(
                            hint="wait_spill", nofuse=True
                        )
                        cur = nc.cur_bb.bb.instructions
                        cur.remove(nop.ins)
                        if nop.ins.sync_info is None:
                            nop.ins.sync_info = mybir.SyncInfo(
                                on_update=[], on_wait=[]
                            )
                        nop.ins.sync_info.on_wait.append(w)
                        insts.insert(i, nop.ins)
                        i += 1
                    i += 1

    def _drain_and_barrier(self, tick_clock, wait_clock):
        nc = self.nc
        drain_inst = nc.sync.drain()
        wait_clock.add_sem_waits(
            drain_inst.ins, ScopedClock({None: tick_clock.global_clock})
        )
        nc.all_engine_barrier()
        assert self.sems is not None
        popped = nc._tile_sem_poison_stack.pop()
        assert popped is self._sem_poison
        nc.clear_and_free_semaphores(list(self.sems.allocated().values()))
        nc.all_engine_barrier()
        _spill_excess_waits(nc)

    tile.TileContext._drain_and_barrier = _drain_and_barrier
    tile.TileContext._drain_patched = True


def _build_program():
    _patch_tail_drain()
    nc = bass.Bass()

    x = nc.dram_tensor("x", [N, C], BF16, kind="ExternalInput")
    # host-prearranged weight layouts (see kernel() below)
    w_q = nc.dram_tensor("w_q", [128, NCJ, HPC, HD], BF16, kind="ExternalInput")
    w_k = nc.dram_tensor("w_k", [128, NCJ, HPC, HD], BF16, kind="ExternalInput")
    w_v = nc.dram_tensor("w_v", [128, NCJ, HPC * HD], BF16, kind="ExternalInput")
    w_o = nc.dram_tensor("w_o", [128, HPC, C], BF16, kind="ExternalInput")
    b_q = nc.dram_tensor("b_q", [HD, HPC], F32, kind="ExternalInput")
    b_k = nc.dram_tensor("b_k", [HD, HPC], F32, kind="ExternalInput")
    out = nc.dram_tensor("out", [N, C], F32, kind="ExternalOutput")

    from contextlib import ExitStack

    with tile.TileContext(nc) as tc, ExitStack() as ctx:
        const = ctx.enter_context(tc.tile_pool(name="const", bufs=1))
        w_q_sb = const.tile([128, NCJ, HPC, HD], BF16)
        w_k_sb = const.tile([128, NCJ, HPC, HD], BF16)
        w_v_sb = const.tile([128, NCJ, HPC * HD], BF16)
        w_o_sb = const.tile([128, HPC, C], BF16)
        b_q_sb = const.tile([HD, HPC], F32)
        b_k_sb = const.tile([HD, HPC], F32)
        ones_row = const.tile([1, HD], F32)
        nc.vector.memset(ones_row, 1.0)
        nc.gpsimd.dma_start(out=w_q_sb, in_=w_q[:])
        nc.gpsimd.dma_start(out=w_k_sb, in_=w_k[:])
        nc.gpsimd.dma_start(out=w_v_sb, in_=w_v[:])
        nc.gpsimd.dma_start(out=w_o_sb, in_=w_o[:])
        nc.gpsimd.dma_start(out=b_q_sb, in_=b_q[:])
        nc.gpsimd.dma_start(out=b_k_sb, in_=b_k[:])

        persist = ctx.enter_context(tc.tile_pool(name="persist", bufs=1))
        # K=128 zero-padded: rows 0:64 hold the head's q/k rows, 64:128 stay
        # zero (K=64 M=128 single-group matmuls run at half rate on this HW)
        qT = persist.tile([128, HPC, N], BF16)
        kT = persist.tile([128, HPC, N], BF16)
        nc.gpsimd.memset(qT[HD:128, :, :], 0.0)
        nc.gpsimd.memset(kT[HD:128, :, :], 0.0)
        # [tokens, head, kb, 128]: dims at 0:64, ones at 64, rest padding --
        # each (h, kb) block starts 256B-aligned for the xbar transpose DMA
        v_nat = persist.tile([128, HPC, NT, 128], BF16)

        # ---- phase 0/1: xT via DMA transpose, qkv projections, v layout ----
        with (
            tc.tile_pool(name="xTp", bufs=1) as xTp,
            tc.tile_pool(name="vTp", bufs=1) as vTp,
            tc.tile_pool(name="ps_m", bufs=3, space="PSUM") as ps_m,
        ):
            xT = xTp.tile([128, NCJ, N], BF16)
            # per-head vT kept at partition offset 0: the SBUF DMA-transpose
            # path reads garbage from partition-offset sources
            vTh = [vTp.tile([HD, N], BF16, name=f"vT{h}") for h in range(HPC)]
            for tb in range(QB):
                tsl = slice(tb * 512, (tb + 1) * 512)
                for cj in range(NCJ):
                    nc.sync.dma_start(
                        out=xT[:, cj, tsl],
                        in_=x[tsl, cj * 128:(cj + 1) * 128],
                        transpose=True,
                    )
            nc.vector.memset(v_nat, 1.0)
            for tb in range(QB):
                tsl = slice(tb * 512, (tb + 1) * 512)
                # v first: the attention PV stream consumes v_nat earliest
                for h in range(HPC):
                    pmv = ps_m.tile([HD, 512], F32, tag="pm")
                    for cj in range(NCJ):
                        nc.tensor.matmul(
                            pmv,
                            lhsT=w_v_sb[:, cj, h * HD:(h + 1) * HD],
                            rhs=xT[:, cj, tsl],
                            start=(cj == 0),
                            stop=(cj == NCJ - 1),
                        )
                    nc.vector.tensor_copy(out=vTh[h][:, tsl], in_=pmv)
                for kb in range(tb * 4, tb * 4 + 4):
                    ksl = slice(kb * 128, (kb + 1) * 128)
                    for h in range(HPC):
                        nc.sync.dma_start(
                            out=v_nat[:, h, kb, 0:HD],
                            in_=vTh[h][:, ksl],
                            transpose=True,
                        )
                for w_sb, b_sb, dst in (
                    (w_k_sb, b_k_sb, kT),
                    (w_q_sb, b_q_sb, qT),
                ):
                    for h in range(HPC):
                        pm = ps_m.tile([HD, 512], F32, tag="pm")
                        for cj in range(NCJ):
                            nc.tensor.matmul(
                                pm,
                                lhsT=w_sb[:, cj, h, :],
                                rhs=xT[:, cj, tsl],
                                start=(cj == 0),
                                stop=(cj == NCJ - 1),
                            )
                        nc.vector.tensor_scalar_add(
                            out=dst[0:HD, h, tsl], in0=pm,
                            scalar1=b_sb[:, h:h + 1],
                        )

        # ---- phase 2/3: attention + projection, software-pipelined ----
        with (
            tc.tile_pool(name="oTp", bufs=1) as oTp,
            tc.tile_pool(name="expp", bufs=4) as expp,
            tc.tile_pool(name="recipp", bufs=2) as recipp,
            tc.tile_pool(name="bcsb", bufs=2) as bcsb,
            tc.tile_pool(name="ostage", bufs=3) as ostage,
            tc.tile_pool(name="ps_s", bufs=2, space="PSUM") as ps_s,
            tc.tile_pool(name="ps_o", bufs=2, space="PSUM") as ps_o,
            tc.tile_pool(name="ps_p", bufs=2, space="PSUM") as ps_p,
        ):
            # oT zero-padded to K=128 (rows 64:128 stay 0; w_o rows there are
            # host-zeroed) so the projection avoids the K=64/M=128 slow path
            oT = oTp.tile([128, HPC, N], BF16)
            nc.gpsimd.memset(oT[HD:128, :, :], 0.0)
            pending_recip = []  # flushed @g1 of the following unit (DVE)
            pending_bc = []     # flushed @g5 (PE bcast + DVE normalize)
            pending_proj = []   # flushed @g7 (PE matmuls)

            def make_tail(po, h, qsl, u):
                state = {}

                def recip():
                    rt = recipp.tile([1, 512], F32, name="rt")
                    nc.vector.reciprocal(out=rt, in_=po[HD:HD + 1, :])
                    state["rt"] = rt

                def bcmult():
                    # broadcast across 64 partitions: ones[1,64].T @ rt[1,512]
                    pbc = ps_p.tile([128, C], F32, name="pp")
                    nc.tensor.matmul(
                        pbc[0:HD, :], lhsT=ones_row, rhs=state["rt"],
                        start=True, stop=True,
                    )
                    bc = bcsb.tile([HD, 512], F32, name="bc")
                    nc.vector.tensor_copy(out=bc, in_=pbc[0:HD, :])
                    nc.vector.tensor_mul(
                        out=oT[0:HD, h, qsl], in0=po[0:HD, :], in1=bc
                    )
                return recip, bcmult

            def make_proj(qb, j):
                def proj():
                    q0 = qb * 512 + j * 128
                    pp = ps_p.tile([128, C], F32, name="pp")
                    for h in range(HPC):
                        nc.tensor.matmul(
                            pp,
                            lhsT=oT[:, h, q0:q0 + 128],
                            rhs=w_o_sb[:, h, :],
                            start=(h == 0),
                            stop=(h == HPC - 1),
                        )
                    ot = ostage.tile([128, C], F32, name="ot")
                    nc.vector.tensor_copy(out=ot, in_=pp)
                    nc.sync.dma_start(out=out[q0:q0 + 128, :], in_=ot)
                return proj

            units = [(qb, h) for qb in range(QB) for h in range(HPC)]

            def s_group(u, g):
                qb, h = units[u]
                qsl = slice(qb * 512, (qb + 1) * 512)
                ks = ps_s.tile([128, GP, 512], F32, name="ks")
                for j in range(GP):
                    kb = g * GP + j
                    nc.tensor.matmul(
                        ks[:, j, :],
                        lhsT=kT[:, h, kb * 128:(kb + 1) * 128],
                        rhs=qT[:, h, qsl],
                        start=True,
                        stop=True,
                    )
                return ks

            # flat (unit, group) pipeline: the scores skew carries across
            # unit boundaries so the PE/ACT streams never drain
            flat = [(u, g) for u in range(len(units)) for g in range(NG)]
            po_tiles = {}
            pend = s_group(*flat[0])
            for i, (u, g) in enumerate(flat):
                ks = pend
                pend = s_group(*flat[i + 1]) if i + 1 < len(flat) else None
                if g == 1:
                    for f in pending_recip:
                        f()
                    pending_recip.clear()
                elif g == 5:
                    for f in pending_bc:
                        f()
                    pending_bc.clear()
                elif g in (7, 9, 11, 13) and pending_proj:
                    pending_proj.pop(0)()
                et = expp.tile([128, GP, 512], BF16)
                nc.scalar.activation(out=et, in_=ks, func=AF.Exp, scale=SCALE)
                qb, h = units[u]
                if g == 0:
                    po_tiles[u] = ps_o.tile([HD + 1, 512], F32, name="po")
                po = po_tiles[u]
                for j in range(GP):
                    kb = g * GP + j
                    nc.tensor.matmul(
                        po,
                        lhsT=v_nat[:, h, kb, 0:HD + 1],
                        rhs=et[:, j, :],
                        start=(kb == 0),
                        stop=(kb == NT - 1),
                    )
                if g == NG - 1:
                    qsl = slice(qb * 512, (qb + 1) * 512)
                    recip, bcmult = make_tail(po_tiles.pop(u), h, qsl, u)
                    pending_recip.append(recip)
                    pending_bc.append(bcmult)
                    if h == HPC - 1:
                        for j in range(4):
                            pending_proj.append(make_proj(qb, j))
            for f in pending_recip:
                f()
            for f in pending_bc:
                f()
            for f in pending_proj:
                f()

    return nc


_PROGRAM = None


def _get_program():
    global _PROGRAM
    if _PROGRAM is None:
        _PROGRAM = _build_program()
    return _PROGRAM


def _bf16(a):
    import ml_dtypes

    return np.asarray(a, dtype=np.float32).astype(ml_dtypes.bfloat16)


def _prep_core_inputs(x, W_qkv, b_qkv, heads, batch):
    """Host-side slicing/relayout for one core."""
    cols = np.concatenate([np.arange(h * HD, (h + 1) * HD) for h in heads])
    w_q = W_qkv[:, cols]               # [512, 128]
    w_k = W_qkv[:, C + cols]
    w_v = W_qkv[:, 2 * C + cols]
    # [512, 128] -> [128 partitions, NCJ chunks, ...]
    w_q = np.ascontiguousarray(
        w_q.reshape(NCJ, 128, HPC, HD).transpose(1, 0, 2, 3))
    w_k = np.ascontiguousarray(
        w_k.reshape(NCJ, 128, HPC, HD).transpose(1, 0, 2, 3))
    w_v = np.ascontiguousarray(
        w_v.reshape(NCJ, 128, HPC * HD).transpose(1, 0, 2))
    b_q = np.ascontiguousarray(b_qkv[cols].reshape(HPC, HD).T)      # [64, 2]
    b_k = np.ascontiguousarray(b_qkv[C + cols].reshape(HPC, HD).T)
    return {
        "x": _bf16(np.ascontiguousarray(x[batch])),
        "w_q": _bf16(w_q),
        "w_k": _bf16(w_k),
        "w_v": _bf16(w_v),
        "b_q": b_q.astype(np.float32),
        "b_k": b_k.astype(np.float32),
    }


def _core_w_o(W_out, heads):
    rows = np.concatenate([np.arange(h * HD, (h + 1) * HD) for h in heads])
    w = np.zeros((128, HPC, C), dtype=np.float32)
    w[0:HD] = W_out[rows].reshape(HPC, HD, C).transpose(1, 0, 2)
    return _bf16(w)


def kernel(x, W_qkv, b_qkv, W_out, b_out):
    x = np.asarray(x, dtype=np.float32)
    W_qkv = np.asarray(W_qkv, dtype=np.float32)
    b_qkv = np.asarray(b_qkv, dtype=np.float32)
    W_out = np.asarray(W_out, dtype=np.float32)
    b_out = np.asarray(b_out, dtype=np.float32)

    nc = _get_program()
    in_maps = []
    for c in range(NCORES):
        batch, hp = c // 4, c % 4
        heads = [2 * hp, 2 * hp + 1]
        im = _prep_core_inputs(x, W_qkv, b_qkv, heads, batch)
        im["w_o"] = _core_w_o(W_out, heads)
        in_maps.append(im)

    res = run_bass_kernel_spmd(nc, in_maps, core_ids=list(range(NCORES)))

    # v-bias commutes: softmax rows sum to 1, so (P @ (V + 1 b_v)) @ W_o
    # = P@V@W_o + b_v@W_o. Add b_v@W_out and b_out once on the host.
    const_row = b_qkv[2 * C:] @ W_out + b_out    # [512]
    out = np.empty((B, N, C), dtype=np.float32)
    for b in range(B):
        acc = res.results[4 * b]["out"].astype(np.float32).copy()
        for c in range(4 * b + 1, 4 * b + 4):
            acc += res.results[c]["out"]
        out[b] = acc + const_row
    return out


# revision 40
# speedup vs baseline: 1.1485x; 1.1485x over previous
"""Multi-head attention (B=2, N=4096, C=512, H=8) on 8 TRN2 NeuronCores.

Sharding: core c handles batch c//4 and heads {2*(c%4), 2*(c%4)+1}
(data parallel over batch, tensor parallel over heads). Each core
computes its 2 heads' attention plus a partial output projection;
the host sums the 4 partials per batch and adds the bias terms
(b_out and b_v @ W_out, which commutes past softmax-weighted sums).

Compute layout per core (matmul operands bf16, accumulation f32):
  xT    = x.T via 2-byte DMA transpose (x pre-cast bf16 on host)
  qT,kT = per-head rows of (x @ Wq + bq).T etc.    [64, 2, N] bf16
  v     = x @ Wv, DMA-transposed to natural layout with a ones column
          per head (ones turns P@[V|1] into [P@V | rowsum(P)])
  per (512-query block, head), groups of 2 key-chunks:
     scoresT = kT_kb.T(stationary) @ qT   -> PSUM [128, 2, 512] f32
     expT    = Exp(scoresT / sqrt(C))     -> SBUF bf16 (one wide ACT op)
     accum P.T @ [v|1] over kb            -> PSUM [65, 512] f32
  tail (deferred one unit so the PE stream never blocks): reciprocal of
  row 64, partition-broadcast via a DRAM round-trip, normalize+cast bf16
  proj (deferred two units): oT.T @ W_out rows (K=64 per head, accum)
"""

import numpy as np

import concourse.bass as bass
import concourse.mybir as mybir
import concourse.tile as tile
from concourse.bass_utils import run_bass_kernel_spmd
from concourse.vector_clock import ScopedClock

F32 = mybir.dt.float32
BF16 = mybir.dt.bfloat16
AF = mybir.ActivationFunctionType

B, N, C, H = 2, 4096, 512, 8
HD = C // H          # 64
HPC = H // 4         # 2 heads per core
NCORES = 8
NT = N // 128        # 32 key chunks
NCJ = C // 128       # 4 contraction chunks
QB = N // 512        # 8 query blocks
GP = 2               # key chunks per exp group (PSUM banks per scores tile)
NG = NT // GP
NUNITS = QB * HPC
SCALE = 1.0 / float(np.sqrt(C))


def _patch_tail_drain():
    """This walrus build caps sync waits at 1 per non-EventSemaphore
    instruction (2 for EventSemaphore); the stock TileContext tail-drain
    attaches every outstanding wait to one Drain, and the scheduler can
    leave >1 wait on regular instructions. Spill extras onto fresh
    same-engine nops inserted just before the over-subscribed one."""
    if getattr(tile.TileContext, "_drain_patched", False):
        return

    def _spill_excess_waits(nc):
        for fn in nc.m.functions:
            for bb in fn.blocks:
                insts = bb.instructions
                i = 0
                while i < len(insts):
                    inst = insts[i]
                    si = inst.sync_info
                    cap = 2 if isinstance(inst, mybir.InstEventSemaphore) else 1
                    if si is None or len(si.on_wait) <= cap:
                        i += 1
                        continue
                    extra = list(si.on_wait[cap:])
                    si.on_wait[:] = si.on_wait[:cap]
                    for w in extra:
                        nop = nc.engines[inst.engine].nop(
                            hint="wait_spill", nofuse=True
                        )
                        cur = nc.cur_bb.bb.instructions
                        cur.remove(nop.ins)
                        if nop.ins.sync_info is None:
                            nop.ins.sync_info = mybir.SyncInfo(
                                on_update=[], on_wait=[]
                            )
                        nop.ins.sync_info.on_wait.append(w)
                        insts.insert(i, nop.ins)
                        i += 1
                    i += 1

    def _drain_and_barrier(self, tick_clock, wait_clock):
        nc = self.nc
        drain_inst = nc.sync.drain()
        wait_clock.add_sem_waits(
            drain_inst.ins, ScopedClock({None: tick_clock.global_clock})
        )
        nc.all_engine_barrier()
        assert self.sems is not None
        popped = nc._tile_sem_poison_stack.pop()
        assert popped is self._sem_poison
        nc.clear_and_free_semaphores(list(self.sems.allocated().values()))
        nc.all_engine_barrier()
        _spill_excess_waits(nc)

    tile.TileContext._drain_and_barrier = _drain_and_barrier
    tile.TileContext._drain_patched = True


def _build_program():
    _patch_tail_drain()
    nc = bass.Bass()

    xt = nc.dram_tensor("xt", [128, NCJ, N], BF16, kind="ExternalInput")
    # host-prearranged weight layouts (see kernel() below)
    w_q = nc.dram_tensor("w_q", [128, NCJ, HPC, HD], BF16, kind="ExternalInput")
    w_k = nc.dram_tensor("w_k", [128, NCJ, HPC, HD], BF16, kind="ExternalInput")
    w_v = nc.dram_tensor("w_v", [128, NCJ, HPC * HD], BF16, kind="ExternalInput")
    w_o = nc.dram_tensor("w_o", [128, HPC, C], BF16, kind="ExternalInput")
    b_q = nc.dram_tensor("b_q", [HD, HPC], F32, kind="ExternalInput")
    b_k = nc.dram_tensor("b_k", [HD, HPC], F32, kind="ExternalInput")
    out = nc.dram_tensor("out", [N, C], F32, kind="ExternalOutput")

    from contextlib import ExitStack

    with tile.TileContext(nc) as tc, ExitStack() as ctx:
        const = ctx.enter_context(tc.tile_pool(name="const", bufs=1))
        w_q_sb = const.tile([128, NCJ, HPC, HD], BF16)
        w_k_sb = const.tile([128, NCJ, HPC, HD], BF16)
        w_v_sb = const.tile([128, NCJ, HPC * HD], BF16)
        w_o_sb = const.tile([128, HPC, C], BF16)
        b_q_sb = const.tile([HD, HPC], F32)
        b_k_sb = const.tile([HD, HPC], F32)
        ones_row = const.tile([1, HD], F32)
        nc.vector.memset(ones_row, 1.0)
        nc.gpsimd.dma_start(out=w_q_sb, in_=w_q[:])
        nc.gpsimd.dma_start(out=w_k_sb, in_=w_k[:])
        nc.gpsimd.dma_start(out=w_v_sb, in_=w_v[:])
        nc.gpsimd.dma_start(out=w_o_sb, in_=w_o[:])
        nc.gpsimd.dma_start(out=b_q_sb, in_=b_q[:])
        nc.gpsimd.dma_start(out=b_k_sb, in_=b_k[:])

        persist = ctx.enter_context(tc.tile_pool(name="persist", bufs=1))
        # K=128 zero-padded: rows 0:64 hold the head's q/k rows, 64:128 stay
        # zero (K=64 M=128 single-group matmuls run at half rate on this HW)
        qT = persist.tile([128, HPC, N], BF16)
        kT = persist.tile([128, HPC, N], BF16)
        nc.gpsimd.memset(qT[HD:128, :, :], 0.0)
        nc.gpsimd.memset(kT[HD:128, :, :], 0.0)
        # [tokens, head, kb, 128]: dims at 0:64, ones at 64, rest padding --
        # each (h, kb) block starts 256B-aligned for the xbar transpose DMA
        v_nat = persist.tile([128, HPC, NT, 128], BF16)

        # ---- phase 0/1: load host-transposed xT, qkv projections ----
        with (
            tc.tile_pool(name="xTp", bufs=1) as xTp,
            tc.tile_pool(name="ps_m", bufs=3, space="PSUM") as ps_m,
            tc.tile_pool(name="ps_v", bufs=3, space="PSUM") as ps_v,
        ):
            xT = xTp.tile([128, NCJ, N], BF16)
            for tb in range(QB):
                tsl = slice(tb * 512, (tb + 1) * 512)
                nc.sync.dma_start(out=xT[:, :, tsl], in_=xt[:, :, tsl])
            nc.vector.memset(v_nat, 1.0)
            for tb in range(QB):
                tsl = slice(tb * 512, (tb + 1) * 512)
                # v first, projected straight into natural layout
                # (lhsT = xT token block, rhs = W_v), no transposes anywhere
                for kb in range(tb * 4, tb * 4 + 4):
                    ksl = slice(kb * 128, (kb + 1) * 128)
                    pv_ = ps_v.tile([128, HPC * HD], F32, name="pv_")
                    for cj in range(NCJ):
                        nc.tensor.matmul(
                            pv_,
                            lhsT=xT[:, cj, ksl],
                            rhs=w_v_sb[:, cj, :],
                            start=(cj == 0),
                            stop=(cj == NCJ - 1),
                        )
                    for h in range(HPC):
                        nc.vector.tensor_copy(
                            out=v_nat[:, h, kb, 0:HD],
                            in_=pv_[:, h * HD:(h + 1) * HD],
                        )
                for w_sb, b_sb, dst in (
                    (w_k_sb, b_k_sb, kT),
                    (w_q_sb, b_q_sb, qT),
                ):
                    for h in range(HPC):
                        pm = ps_m.tile([HD, 512], F32, tag="pm")
                        for cj in range(NCJ):
                            nc.tensor.matmul(
                                pm,
                                lhsT=w_sb[:, cj, h, :],
                                rhs=xT[:, cj, tsl],
                                start=(cj == 0),
                                stop=(cj == NCJ - 1),
                            )
                        nc.vector.tensor_scalar_add(
                            out=dst[0:HD, h, tsl], in0=pm,
                            scalar1=b_sb[:, h:h + 1],
                        )

        # ---- phase 2/3: attention + projection, software-pipelined ----
        with (
            tc.tile_pool(name="oTp", bufs=1) as oTp,
            tc.tile_pool(name="expp", bufs=4) as expp,
            tc.tile_pool(name="recipp", bufs=2) as recipp,
            tc.tile_pool(name="bcsb", bufs=2) as bcsb,
            tc.tile_pool(name="ostage", bufs=3) as ostage,
            tc.tile_pool(name="ps_s", bufs=2, space="PSUM") as ps_s,
            tc.tile_pool(name="ps_o", bufs=2, space="PSUM") as ps_o,
            tc.tile_pool(name="ps_p", bufs=2, space="PSUM") as ps_p,
        ):
            # oT zero-padded to K=128 (rows 64:128 stay 0; w_o rows there are
            # host-zeroed) so the projection avoids the K=64/M=128 slow path
            oT = oTp.tile([128, HPC, N], BF16)
            nc.gpsimd.memset(oT[HD:128, :, :], 0.0)
            pending_recip = []  # flushed @g1 of the following unit (DVE)
            pending_bc = []     # flushed @g5 (PE bcast + DVE normalize)
            pending_proj = []   # flushed @g7 (PE matmuls)

            def make_tail(po, h, qsl, u):
                state = {}

                def recip():
                    rt = recipp.tile([1, 512], F32, name="rt")
                    nc.vector.reciprocal(out=rt, in_=po[HD:HD + 1, :])
                    state["rt"] = rt

                def bcmult():
                    # broadcast across 64 partitions: ones[1,64].T @ rt[1,512]
                    pbc = ps_p.tile([128, C], F32, name="pp")
                    nc.tensor.matmul(
                        pbc[0:HD, :], lhsT=ones_row, rhs=state["rt"],
                        start=True, stop=True,
                    )
                    bc = bcsb.tile([HD, 512], F32, name="bc")
                    nc.vector.tensor_copy(out=bc, in_=pbc[0:HD, :])
                    nc.vector.tensor_mul(
                        out=oT[0:HD, h, qsl], in0=po[0:HD, :], in1=bc
                    )
                return recip, bcmult

            def make_proj(qb, j):
                def proj():
                    q0 = qb * 512 + j * 128
                    pp = ps_p.tile([128, C], F32, name="pp")
                    for h in range(HPC):
                        nc.tensor.matmul(
                            pp,
                            lhsT=oT[:, h, q0:q0 + 128],
                            rhs=w_o_sb[:, h, :],
                            start=(h == 0),
                            stop=(h == HPC - 1),
                        )
                    ot = ostage.tile([128, C], F32, name="ot")
                    nc.vector.tensor_copy(out=ot, in_=pp)
                    nc.sync.dma_start(out=out[q0:q0 + 128, :], in_=ot)
                return proj

            units = [(qb, h) for qb in range(QB) for h in range(HPC)]

            def s_group(u, g):
                qb, h = units[u]
                qsl = slice(qb * 512, (qb + 1) * 512)
                ks = ps_s.tile([128, GP, 512], F32, name="ks")
                for j in range(GP):
                    kb = g * GP + j
                    nc.tensor.matmul(
                        ks[:, j, :],
                        lhsT=kT[:, h, kb * 128:(kb + 1) * 128],
                        rhs=qT[:, h, qsl],
                        start=True,
                        stop=True,
                    )
                return ks

            # flat (unit, group) pipeline: the scores skew carries across
            # unit boundaries so the PE/ACT streams never drain
            flat = [(u, g) for u in range(len(units)) for g in range(NG)]
            po_tiles = {}
            pend = s_group(*flat[0])
            for i, (u, g) in enumerate(flat):
                ks = pend
                pend = s_group(*flat[i + 1]) if i + 1 < len(flat) else None
                if g == 1:
                    for f in pending_recip:
                        f()
                    pending_recip.clear()
                elif g == 5:
                    for f in pending_bc:
                        f()
                    pending_bc.clear()
                elif g in (7, 9, 11, 13) and pending_proj:
                    pending_proj.pop(0)()
                et = expp.tile([128, GP, 512], BF16)
                nc.scalar.activation(out=et, in_=ks, func=AF.Exp, scale=SCALE)
                qb, h = units[u]
                if g == 0:
                    po_tiles[u] = ps_o.tile([HD + 1, 512], F32, name="po")
                po = po_tiles[u]
                for j in range(GP):
                    kb = g * GP + j
                    nc.tensor.matmul(
                        po,
                        lhsT=v_nat[:, h, kb, 0:HD + 1],
                        rhs=et[:, j, :],
                        start=(kb == 0),
                        stop=(kb == NT - 1),
                    )
                if g == NG - 1:
                    qsl = slice(qb * 512, (qb + 1) * 512)
                    recip, bcmult = make_tail(po_tiles.pop(u), h, qsl, u)
                    pending_recip.append(recip)
                    pending_bc.append(bcmult)
                    if h == HPC - 1:
                        for j in range(4):
                            pending_proj.append(make_proj(qb, j))
            for f in pending_recip:
                f()
            for f in pending_bc:
                f()
            for f in pending_proj:
                f()

    return nc


_PROGRAM = None


def _get_program():
    global _PROGRAM
    if _PROGRAM is None:
        _PROGRAM = _build_program()
    return _PROGRAM


def _bf16(a):
    import ml_dtypes

    return np.asarray(a, dtype=np.float32).astype(ml_dtypes.bfloat16)


def _prep_core_inputs(x, W_qkv, b_qkv, heads, batch):
    """Host-side slicing/relayout for one core."""
    cols = np.concatenate([np.arange(h * HD, (h + 1) * HD) for h in heads])
    w_q = W_qkv[:, cols]               # [512, 128]
    w_k = W_qkv[:, C + cols]
    w_v = W_qkv[:, 2 * C + cols]
    # [512, 128] -> [128 partitions, NCJ chunks, ...]
    w_q = np.ascontiguousarray(
        w_q.reshape(NCJ, 128, HPC, HD).transpose(1, 0, 2, 3))
    w_k = np.ascontiguousarray(
        w_k.reshape(NCJ, 128, HPC, HD).transpose(1, 0, 2, 3))
    w_v = np.ascontiguousarray(
        w_v.reshape(NCJ, 128, HPC * HD).transpose(1, 0, 2))
    b_q = np.ascontiguousarray(b_qkv[cols].reshape(HPC, HD).T)      # [64, 2]
    b_k = np.ascontiguousarray(b_qkv[C + cols].reshape(HPC, HD).T)
    xt = np.ascontiguousarray(
        x[batch].T.reshape(NCJ, 128, N).transpose(1, 0, 2))
    return {
        "xt": _bf16(xt),
        "w_q": _bf16(w_q),
        "w_k": _bf16(w_k),
        "w_v": _bf16(w_v),
        "b_q": b_q.astype(np.float32),
        "b_k": b_k.astype(np.float32),
    }


def _core_w_o(W_out, heads):
    rows = np.concatenate([np.arange(h * HD, (h + 1) * HD) for h in heads])
    w = np.zeros((128, HPC, C), dtype=np.float32)
    w[0:HD] = W_out[rows].reshape(HPC, HD, C).transpose(1, 0, 2)
    return _bf16(w)


def kernel(x, W_qkv, b_qkv, W_out, b_out):
    x = np.asarray(x, dtype=np.float32)
    W_qkv = np.asarray(W_qkv, dtype=np.float32)
    b_qkv = np.asarray(b_qkv, dtype=np.float32)
    W_out = np.asarray(W_out, dtype=np.float32)
    b_out = np.asarray(b_out, dtype=np.float32)

    nc = _get_program()
    in_maps = []
    for c in range(NCORES):
        batch, hp = c // 4, c % 4
        heads = [2 * hp, 2 * hp + 1]
        im = _prep_core_inputs(x, W_qkv, b_qkv, heads, batch)
        im["w_o"] = _core_w_o(W_out, heads)
        in_maps.append(im)

    res = run_bass_kernel_spmd(nc, in_maps, core_ids=list(range(NCORES)))

    # v-bias commutes: softmax rows sum to 1, so (P @ (V + 1 b_v)) @ W_o
    # = P@V@W_o + b_v@W_o. Add b_v@W_out and b_out once on the host.
    const_row = b_qkv[2 * C:] @ W_out + b_out    # [512]
    out = np.empty((B, N, C), dtype=np.float32)
    for b in range(B):
        acc = res.results[4 * b]["out"].astype(np.float32).copy()
        for c in range(4 * b + 1, 4 * b + 4):
            acc += res.results[c]["out"]
        out[b] = acc + const_row
    return out


# revision 43
# speedup vs baseline: 1.2014x; 1.0460x over previous
"""Multi-head attention (B=2, N=4096, C=512, H=8) on 8 TRN2 NeuronCores.

Sharding: core c handles batch c//4 and heads {2*(c%4), 2*(c%4)+1}
(data parallel over batch, tensor parallel over heads). Each core
computes its 2 heads' attention plus a partial output projection;
the host sums the 4 partials per batch and adds the bias terms
(b_out and b_v @ W_out, which commutes past softmax-weighted sums).

Compute layout per core (matmul operands bf16, accumulation f32):
  xT    = x.T via 2-byte DMA transpose (x pre-cast bf16 on host)
  qT,kT = per-head rows of (x @ Wq + bq).T etc.    [64, 2, N] bf16
  v     = x @ Wv, DMA-transposed to natural layout with a ones column
          per head (ones turns P@[V|1] into [P@V | rowsum(P)])
  per (512-query block, head), groups of 2 key-chunks:
     scoresT = kT_kb.T(stationary) @ qT   -> PSUM [128, 2, 512] f32
     expT    = Exp(scoresT / sqrt(C))     -> SBUF bf16 (one wide ACT op)
     accum P.T @ [v|1] over kb            -> PSUM [65, 512] f32
  tail (deferred one unit so the PE stream never blocks): reciprocal of
  row 64, partition-broadcast via a DRAM round-trip, normalize+cast bf16
  proj (deferred two units): oT.T @ W_out rows (K=64 per head, accum)
"""

import numpy as np

import concourse.bass as bass
import concourse.mybir as mybir
import concourse.tile as tile
from concourse.bass_utils import run_bass_kernel_spmd
from concourse.vector_clock import ScopedClock

F32 = mybir.dt.float32
BF16 = mybir.dt.bfloat16
AF = mybir.ActivationFunctionType

B, N, C, H = 2, 4096, 512, 8
HD = C // H          # 64
HPC = H // 4         # 2 heads per core
NCORES = 8
NT = N // 128        # 32 key chunks
NCJ = C // 128       # 4 contraction chunks
QB = N // 512        # 8 query blocks
GP = 2               # key chunks per exp group (PSUM banks per scores tile)
NG = NT // GP
NUNITS = QB * HPC
SCALE = 1.0 / float(np.sqrt(C))


def _patch_tail_drain():
    """This walrus build caps sync waits at 1 per non-EventSemaphore
    instruction (2 for EventSemaphore); the stock TileContext tail-drain
    attaches every outstanding wait to one Drain, and the scheduler can
    leave >1 wait on regular instructions. Spill extras onto fresh
    same-engine nops inserted just before the over-subscribed one."""
    if getattr(tile.TileContext, "_drain_patched", False):
        return

    def _spill_excess_waits(nc):
        for fn in nc.m.functions:
            for bb in fn.blocks:
                insts = bb.instructions
                i = 0
                while i < len(insts):
                    inst = insts[i]
                    si = inst.sync_info
                    cap = 2 if isinstance(inst, mybir.InstEventSemaphore) else 1
                    if si is None or len(si.on_wait) <= cap:
                        i += 1
                        continue
                    extra = list(si.on_wait[cap:])
                    si.on_wait[:] = si.on_wait[:cap]
                    for w in extra:
                        nop = nc.engines[inst.engine].nop(
                            hint="wait_spill", nofuse=True
                        )
                        cur = nc.cur_bb.bb.instructions
                        cur.remove(nop.ins)
                        if nop.ins.sync_info is None:
                            nop.ins.sync_info = mybir.SyncInfo(
                                on_update=[], on_wait=[]
                            )
                        nop.ins.sync_info.on_wait.append(w)
                        insts.insert(i, nop.ins)
                        i += 1
                    i += 1

    def _drain_and_barrier(self, tick_clock, wait_clock):
        nc = self.nc
        drain_inst = nc.sync.drain()
        wait_clock.add_sem_waits(
            drain_inst.ins, ScopedClock({None: tick_clock.global_clock})
        )
        nc.all_engine_barrier()
        assert self.sems is not None
        popped = nc._tile_sem_poison_stack.pop()
        assert popped is self._sem_poison
        nc.clear_and_free_semaphores(list(self.sems.allocated().values()))
        nc.all_engine_barrier()
        _spill_excess_waits(nc)

    tile.TileContext._drain_and_barrier = _drain_and_barrier
    tile.TileContext._drain_patched = True


def _build_program():
    _patch_tail_drain()
    nc = bass.Bass()

    xt = nc.dram_tensor("xt", [128, NCJ, N], BF16, kind="ExternalInput")
    # host-prearranged weight layouts (see kernel() below)
    w_q = nc.dram_tensor("w_q", [128, NCJ, HPC, HD], BF16, kind="ExternalInput")
    w_k = nc.dram_tensor("w_k", [128, NCJ, HPC, HD], BF16, kind="ExternalInput")
    w_v = nc.dram_tensor("w_v", [128, NCJ, HPC * HD], BF16, kind="ExternalInput")
    w_o = nc.dram_tensor("w_o", [128, HPC, C], BF16, kind="ExternalInput")
    b_q = nc.dram_tensor("b_q", [HD, HPC], F32, kind="ExternalInput")
    b_k = nc.dram_tensor("b_k", [HD, HPC], F32, kind="ExternalInput")
    out = nc.dram_tensor("out", [N, C], F32, kind="ExternalOutput")

    from contextlib import ExitStack

    with tile.TileContext(nc) as tc, ExitStack() as ctx:
        const = ctx.enter_context(tc.tile_pool(name="const", bufs=1))
        w_q_sb = const.tile([128, NCJ, HPC, HD], BF16)
        w_k_sb = const.tile([128, NCJ, HPC, HD], BF16)
        w_v_sb = const.tile([128, NCJ, HPC * HD], BF16)
        w_o_sb = const.tile([128, HPC, C], BF16)
        b_q_sb = const.tile([HD, HPC], F32)
        b_k_sb = const.tile([HD, HPC], F32)
        # e0 selector: column m of lhsT is e0 -> out[m, :] = rhs row 0
        # (K=128 broadcast matmul; K=1/K=64-with-M=128 shapes run half-rate)
        e0_mat = const.tile([128, HD], F32)
        nc.vector.memset(e0_mat, 0.0)
        nc.vector.memset(e0_mat[0:1, :], 1.0)
        nc.gpsimd.dma_start(out=w_q_sb, in_=w_q[:])
        nc.gpsimd.dma_start(out=w_k_sb, in_=w_k[:])
        nc.gpsimd.dma_start(out=w_v_sb, in_=w_v[:])
        nc.gpsimd.dma_start(out=w_o_sb, in_=w_o[:])
        nc.gpsimd.dma_start(out=b_q_sb, in_=b_q[:])
        nc.gpsimd.dma_start(out=b_k_sb, in_=b_k[:])

        persist = ctx.enter_context(tc.tile_pool(name="persist", bufs=1))
        # K=128 zero-padded: rows 0:64 hold the head's q/k rows, 64:128 stay
        # zero (K=64 M=128 single-group matmuls run at half rate on this HW)
        qT = persist.tile([128, HPC, N], BF16)
        kT = persist.tile([128, HPC, N], BF16)
        nc.gpsimd.memset(qT[HD:128, :, :], 0.0)
        nc.gpsimd.memset(kT[HD:128, :, :], 0.0)
        # [tokens, head, kb, 128]: dims at 0:64, ones at 64, rest padding --
        # each (h, kb) block starts 256B-aligned for the xbar transpose DMA
        v_nat = persist.tile([128, HPC, NT, 128], BF16)

        # ---- phase 0/1: load host-transposed xT, qkv projections ----
        with (
            tc.tile_pool(name="xTp", bufs=1) as xTp,
            tc.tile_pool(name="ps_m", bufs=3, space="PSUM") as ps_m,
            tc.tile_pool(name="ps_v", bufs=3, space="PSUM") as ps_v,
        ):
            xT = xTp.tile([128, NCJ, N], BF16)
            for tb in range(QB):
                tsl = slice(tb * 512, (tb + 1) * 512)
                nc.sync.dma_start(out=xT[:, :, tsl], in_=xt[:, :, tsl])
            nc.vector.memset(v_nat, 1.0)
            for tb in range(QB):
                tsl = slice(tb * 512, (tb + 1) * 512)
                # v first, projected straight into natural layout
                # (lhsT = xT token block, rhs = W_v), no transposes anywhere
                for kb in range(tb * 4, tb * 4 + 4):
                    ksl = slice(kb * 128, (kb + 1) * 128)
                    pv_ = ps_v.tile([128, HPC * HD], F32, name="pv_")
                    for cj in range(NCJ):
                        nc.tensor.matmul(
                            pv_,
                            lhsT=xT[:, cj, ksl],
                            rhs=w_v_sb[:, cj, :],
                            start=(cj == 0),
                            stop=(cj == NCJ - 1),
                        )
                    for h in range(HPC):
                        nc.vector.tensor_copy(
                            out=v_nat[:, h, kb, 0:HD],
                            in_=pv_[:, h * HD:(h + 1) * HD],
                        )
                for w_sb, b_sb, dst in (
                    (w_k_sb, b_k_sb, kT),
                    (w_q_sb, b_q_sb, qT),
                ):
                    for h in range(HPC):
                        pm = ps_m.tile([HD, 512], F32, tag="pm")
                        for cj in range(NCJ):
                            nc.tensor.matmul(
                                pm,
                                lhsT=w_sb[:, cj, h, :],
                                rhs=xT[:, cj, tsl],
                                start=(cj == 0),
                                stop=(cj == NCJ - 1),
                            )
                        nc.vector.tensor_scalar_add(
                            out=dst[0:HD, h, tsl], in0=pm,
                            scalar1=b_sb[:, h:h + 1],
                        )

        # ---- phase 2/3: attention + projection, software-pipelined ----
        with (
            tc.tile_pool(name="oTp", bufs=1) as oTp,
            tc.tile_pool(name="expp", bufs=4) as expp,
            tc.tile_pool(name="recipp", bufs=2) as recipp,
            tc.tile_pool(name="bcsb", bufs=2) as bcsb,
            tc.tile_pool(name="ostage", bufs=3) as ostage,
            tc.tile_pool(name="ps_s", bufs=2, space="PSUM") as ps_s,
            tc.tile_pool(name="ps_o", bufs=2, space="PSUM") as ps_o,
            tc.tile_pool(name="ps_p", bufs=2, space="PSUM") as ps_p,
        ):
            # oT zero-padded to K=128 (rows 64:128 stay 0; w_o rows there are
            # host-zeroed) so the projection avoids the K=64/M=128 slow path
            oT = oTp.tile([128, HPC, N], BF16)
            nc.gpsimd.memset(oT[HD:128, :, :], 0.0)
            pending_recip = []  # flushed @g1 of the following unit (DVE)
            pending_bc = []     # flushed @g5 (PE bcast + DVE normalize)
            pending_proj = []   # flushed @g7 (PE matmuls)

            def make_tail(po, h, qsl, u):
                state = {}

                def recip():
                    # rows 1:128 zeroed so the K=128 broadcast matmul below
                    # multiplies them by e0's zeros without NaN risk
                    rt = recipp.tile([128, 512], F32, name="rt")
                    nc.vector.memset(rt, 0.0)
                    nc.vector.reciprocal(out=rt[0:1, :], in_=po[HD:HD + 1, :])
                    state["rt"] = rt

                def bcmult():
                    # broadcast across 64 partitions: e0[128,64].T @ rt[128,512]
                    pbc = ps_p.tile([128, C], F32, name="pp")
                    nc.tensor.matmul(
                        pbc[0:HD, :], lhsT=e0_mat, rhs=state["rt"],
                        start=True, stop=True,
                    )
                    bc = bcsb.tile([HD, 512], F32, name="bc")
                    nc.vector.tensor_copy(out=bc, in_=pbc[0:HD, :])
                    nc.vector.tensor_mul(
                        out=oT[0:HD, h, qsl], in0=po[0:HD, :], in1=bc
                    )
                return recip, bcmult

            def make_proj(qb, j):
                def proj():
                    q0 = qb * 512 + j * 128
                    pp = ps_p.tile([128, C], F32, name="pp")
                    for h in range(HPC):
                        nc.tensor.matmul(
                            pp,
                            lhsT=oT[:, h, q0:q0 + 128],
                            rhs=w_o_sb[:, h, :],
                            start=(h == 0),
                            stop=(h == HPC - 1),
                        )
                    ot = ostage.tile([128, C], F32, name="ot")
                    nc.vector.tensor_copy(out=ot, in_=pp)
                    nc.sync.dma_start(out=out[q0:q0 + 128, :], in_=ot)
                return proj

            units = [(qb, h) for qb in range(QB) for h in range(HPC)]

            def s_group(u, g):
                qb, h = units[u]
                qsl = slice(qb * 512, (qb + 1) * 512)
                ks = ps_s.tile([128, GP, 512], F32, name="ks")
                for j in range(GP):
                    kb = g * GP + j
                    nc.tensor.matmul(
                        ks[:, j, :],
                        lhsT=kT[:, h, kb * 128:(kb + 1) * 128],
                        rhs=qT[:, h, qsl],
                        start=True,
                        stop=True,
                    )
                return ks

            # flat (unit, group) pipeline: the scores skew carries across
            # unit boundaries so the PE/ACT streams never drain
            flat = [(u, g) for u in range(len(units)) for g in range(NG)]
            po_tiles = {}
            pend = s_group(*flat[0])
            for i, (u, g) in enumerate(flat):
                ks = pend
                pend = s_group(*flat[i + 1]) if i + 1 < len(flat) else None
                if g == 1:
                    for f in pending_recip:
                        f()
                    pending_recip.clear()
                elif g == 5:
                    for f in pending_bc:
                        f()
                    pending_bc.clear()
                elif g in (7, 9, 11, 13) and pending_proj:
                    pending_proj.pop(0)()
                et = expp.tile([128, GP, 512], BF16)
                nc.scalar.activation(out=et, in_=ks, func=AF.Exp, scale=SCALE)
                qb, h = units[u]
                if g == 0:
                    po_tiles[u] = ps_o.tile([HD + 1, 512], F32, name="po")
                po = po_tiles[u]
                for j in range(GP):
                    kb = g * GP + j
                    nc.tensor.matmul(
                        po,
                        lhsT=v_nat[:, h, kb, 0:HD + 1],
                        rhs=et[:, j, :],
                        start=(kb == 0),
                        stop=(kb == NT - 1),
                    )
                if g == NG - 1:
                    qsl = slice(qb * 512, (qb + 1) * 512)
                    recip, bcmult = make_tail(po_tiles.pop(u), h, qsl, u)
                    pending_recip.append(recip)
                    pending_bc.append(bcmult)
                    if h == HPC - 1:
                        for j in range(4):
                            pending_proj.append(make_proj(qb, j))
            for f in pending_recip:
                f()
            for f in pending_bc:
                f()
            for f in pending_proj:
                f()

    return nc


_PROGRAM = None


def _get_program():
    global _PROGRAM
    if _PROGRAM is None:
        _PROGRAM = _build_program()
    return _PROGRAM


def _bf16(a):
    import ml_dtypes

    return np.asarray(a, dtype=np.float32).astype(ml_dtypes.bfloat16)


def _prep_core_inputs(x, W_qkv, b_qkv, heads, batch):
    """Host-side slicing/relayout for one core."""
    cols = np.concatenate([np.arange(h * HD, (h + 1) * HD) for h in heads])
    w_q = W_qkv[:, cols]               # [512, 128]
    w_k = W_qkv[:, C + cols]
    w_v = W_qkv[:, 2 * C + cols]
    # [512, 128] -> [128 partitions, NCJ chunks, ...]
    w_q = np.ascontiguousarray(
        w_q.reshape(NCJ, 128, HPC, HD).transpose(1, 0, 2, 3))
    w_k = np.ascontiguousarray(
        w_k.reshape(NCJ, 128, HPC, HD).transpose(1, 0, 2, 3))
    w_v = np.ascontiguousarray(
        w_v.reshape(NCJ, 128, HPC * HD).transpose(1, 0, 2))
    b_q = np.ascontiguousarray(b_qkv[cols].reshape(HPC, HD).T)      # [64, 2]
    b_k = np.ascontiguousarray(b_qkv[C + cols].reshape(HPC, HD).T)
    xt = np.ascontiguousarray(
        x[batch].T.reshape(NCJ, 128, N).transpose(1, 0, 2))
    return {
        "xt": _bf16(xt),
        "w_q": _bf16(w_q),
        "w_k": _bf16(w_k),
        "w_v": _bf16(w_v),
        "b_q": b_q.astype(np.float32),
        "b_k": b_k.astype(np.float32),
    }


def _core_w_o(W_out, heads):
    rows = np.concatenate([np.arange(h * HD, (h + 1) * HD) for h in heads])
    w = np.zeros((128, HPC, C), dtype=np.float32)
    w[0:HD] = W_out[rows].reshape(HPC, HD, C).transpose(1, 0, 2)
    return _bf16(w)


def kernel(x, W_qkv, b_qkv, W_out, b_out):
    x = np.asarray(x, dtype=np.float32)
    W_qkv = np.asarray(W_qkv, dtype=np.float32)
    b_qkv = np.asarray(b_qkv, dtype=np.float32)
    W_out = np.asarray(W_out, dtype=np.float32)
    b_out = np.asarray(b_out, dtype=np.float32)

    nc = _get_program()
    in_maps = []
    for c in range(NCORES):
        batch, hp = c // 4, c % 4
        heads = [2 * hp, 2 * hp + 1]
        im = _prep_core_inputs(x, W_qkv, b_qkv, heads, batch)
        im["w_o"] = _core_w_o(W_out, heads)
        in_maps.append(im)

    res = run_bass_kernel_spmd(nc, in_maps, core_ids=list(range(NCORES)))

    # v-bias commutes: softmax rows sum to 1, so (P @ (V + 1 b_v)) @ W_o
    # = P@V@W_o + b_v@W_o. Add b_v@W_out and b_out once on the host.
    const_row = b_qkv[2 * C:] @ W_out + b_out    # [512]
    out = np.empty((B, N, C), dtype=np.float32)
    for b in range(B):
        acc = res.results[4 * b]["out"].astype(np.float32).copy()
        for c in range(4 * b + 1, 4 * b + 4):
            acc += res.results[c]["out"]
        out[b] = acc + const_row
    return out


# revision 44
# speedup vs baseline: 1.2021x; 1.0006x over previous
"""Multi-head attention (B=2, N=4096, C=512, H=8) on 8 TRN2 NeuronCores.

Sharding: core c handles batch c//4 and heads {2*(c%4), 2*(c%4)+1}
(data parallel over batch, tensor parallel over heads). Each core
computes its 2 heads' attention plus a partial output projection;
the host sums the 4 partials per batch and adds the bias terms
(b_out and b_v @ W_out, which commutes past softmax-weighted sums).

Compute layout per core (matmul operands bf16, accumulation f32):
  xT    = x.T via 2-byte DMA transpose (x pre-cast bf16 on host)
  qT,kT = per-head rows of (x @ Wq + bq).T etc.    [64, 2, N] bf16
  v     = x @ Wv, DMA-transposed to natural layout with a ones column
          per head (ones turns P@[V|1] into [P@V | rowsum(P)])
  per (512-query block, head), groups of 2 key-chunks:
     scoresT = kT_kb.T(stationary) @ qT   -> PSUM [128, 2, 512] f32
     expT    = Exp(scoresT / sqrt(C))     -> SBUF bf16 (one wide ACT op)
     accum P.T @ [v|1] over kb            -> PSUM [65, 512] f32
  tail (deferred one unit so the PE stream never blocks): reciprocal of
  row 64, partition-broadcast via a DRAM round-trip, normalize+cast bf16
  proj (deferred two units): oT.T @ W_out rows (K=64 per head, accum)
"""

import numpy as np

import concourse.bass as bass
import concourse.mybir as mybir
import concourse.tile as tile
from concourse.bass_utils import run_bass_kernel_spmd
from concourse.vector_clock import ScopedClock

F32 = mybir.dt.float32
BF16 = mybir.dt.bfloat16
AF = mybir.ActivationFunctionType

B, N, C, H = 2, 4096, 512, 8
HD = C // H          # 64
HPC = H // 4         # 2 heads per core
NCORES = 8
NT = N // 128        # 32 key chunks
NCJ = C // 128       # 4 contraction chunks
QB = N // 512        # 8 query blocks
GP = 2               # key chunks per exp group (PSUM banks per scores tile)
NG = NT // GP
NUNITS = QB * HPC
SCALE = 1.0 / float(np.sqrt(C))


def _patch_tail_drain():
    """This walrus build caps sync waits at 1 per non-EventSemaphore
    instruction (2 for EventSemaphore); the stock TileContext tail-drain
    attaches every outstanding wait to one Drain, and the scheduler can
    leave >1 wait on regular instructions. Spill extras onto fresh
    same-engine nops inserted just before the over-subscribed one."""
    if getattr(tile.TileContext, "_drain_patched", False):
        return

    def _spill_excess_waits(nc):
        for fn in nc.m.functions:
            for bb in fn.blocks:
                insts = bb.instructions
                i = 0
                while i < len(insts):
                    inst = insts[i]
                    si = inst.sync_info
                    cap = 2 if isinstance(inst, mybir.InstEventSemaphore) else 1
                    if si is None or len(si.on_wait) <= cap:
                        i += 1
                        continue
                    extra = list(si.on_wait[cap:])
                    si.on_wait[:] = si.on_wait[:cap]
                    for w in extra:
                        nop = nc.engines[inst.engine].nop(
                            hint="wait_spill", nofuse=True
                        )
                        cur = nc.cur_bb.bb.instructions
                        cur.remove(nop.ins)
                        if nop.ins.sync_info is None:
                            nop.ins.sync_info = mybir.SyncInfo(
                                on_update=[], on_wait=[]
                            )
                        nop.ins.sync_info.on_wait.append(w)
                        insts.insert(i, nop.ins)
                        i += 1
                    i += 1

    def _drain_and_barrier(self, tick_clock, wait_clock):
        nc = self.nc
        drain_inst = nc.sync.drain()
        wait_clock.add_sem_waits(
            drain_inst.ins, ScopedClock({None: tick_clock.global_clock})
        )
        nc.all_engine_barrier()
        assert self.sems is not None
        popped = nc._tile_sem_poison_stack.pop()
        assert popped is self._sem_poison
        nc.clear_and_free_semaphores(list(self.sems.allocated().values()))
        nc.all_engine_barrier()
        _spill_excess_waits(nc)

    tile.TileContext._drain_and_barrier = _drain_and_barrier
    tile.TileContext._drain_patched = True


def _build_program():
    _patch_tail_drain()
    nc = bass.Bass()

    xt = nc.dram_tensor("xt", [128, NCJ, N], BF16, kind="ExternalInput")
    # host-prearranged weight layouts (see kernel() below)
    w_q = nc.dram_tensor("w_q", [128, NCJ, HPC, HD], BF16, kind="ExternalInput")
    w_k = nc.dram_tensor("w_k", [128, NCJ, HPC, HD], BF16, kind="ExternalInput")
    w_v = nc.dram_tensor("w_v", [128, NCJ, HPC * HD], BF16, kind="ExternalInput")
    w_o = nc.dram_tensor("w_o", [128, HPC, C], BF16, kind="ExternalInput")
    b_q = nc.dram_tensor("b_q", [HD, HPC], F32, kind="ExternalInput")
    b_k = nc.dram_tensor("b_k", [HD, HPC], F32, kind="ExternalInput")
    out = nc.dram_tensor("out", [N, C], F32, kind="ExternalOutput")

    from contextlib import ExitStack

    with tile.TileContext(nc) as tc, ExitStack() as ctx:
        const = ctx.enter_context(tc.tile_pool(name="const", bufs=1))
        w_q_sb = const.tile([128, NCJ, HPC, HD], BF16)
        w_k_sb = const.tile([128, NCJ, HPC, HD], BF16)
        w_v_sb = const.tile([128, NCJ, HPC * HD], BF16)
        w_o_sb = const.tile([128, HPC, C], BF16)
        b_q_sb = const.tile([HD, HPC], F32)
        b_k_sb = const.tile([HD, HPC], F32)
        # e0 selector: column m of lhsT is e0 -> out[m, :] = rhs row 0
        # (K=128 broadcast matmul; K=1/K=64-with-M=128 shapes run half-rate)
        e0_mat = const.tile([128, HD], F32)
        nc.vector.memset(e0_mat, 0.0)
        nc.vector.memset(e0_mat[0:1, :], 1.0)
        nc.gpsimd.dma_start(out=w_q_sb, in_=w_q[:])
        nc.gpsimd.dma_start(out=w_k_sb, in_=w_k[:])
        nc.gpsimd.dma_start(out=w_v_sb, in_=w_v[:])
        nc.gpsimd.dma_start(out=w_o_sb, in_=w_o[:])
        nc.gpsimd.dma_start(out=b_q_sb, in_=b_q[:])
        nc.gpsimd.dma_start(out=b_k_sb, in_=b_k[:])

        persist = ctx.enter_context(tc.tile_pool(name="persist", bufs=1))
        # K=128 zero-padded: rows 0:64 hold the head's q/k rows, 64:128 stay
        # zero (K=64 M=128 single-group matmuls run at half rate on this HW)
        qT = persist.tile([128, HPC, N], BF16)
        kT = persist.tile([128, HPC, N], BF16)
        nc.gpsimd.memset(qT[HD:128, :, :], 0.0)
        nc.gpsimd.memset(kT[HD:128, :, :], 0.0)
        # [tokens, head, kb, 128]: dims at 0:64, ones at 64, rest padding --
        # each (h, kb) block starts 256B-aligned for the xbar transpose DMA
        v_nat = persist.tile([128, HPC, NT, 128], BF16)

        # ---- phase 0/1: load host-transposed xT, qkv projections ----
        with (
            tc.tile_pool(name="xTp", bufs=1) as xTp,
            tc.tile_pool(name="ps_m", bufs=3, space="PSUM") as ps_m,
            tc.tile_pool(name="ps_v", bufs=3, space="PSUM") as ps_v,
        ):
            xT = xTp.tile([128, NCJ, N], BF16)
            for tb in range(QB):
                tsl = slice(tb * 512, (tb + 1) * 512)
                nc.sync.dma_start(out=xT[:, :, tsl], in_=xt[:, :, tsl])
            nc.vector.memset(v_nat, 1.0)
            for tb in range(QB):
                tsl = slice(tb * 512, (tb + 1) * 512)
                # v first, projected straight into natural layout
                # (lhsT = xT token block, rhs = W_v), no transposes anywhere
                for kb in range(tb * 4, tb * 4 + 4):
                    ksl = slice(kb * 128, (kb + 1) * 128)
                    pv_ = ps_v.tile([128, HPC * HD], F32, name="pv_")
                    for cj in range(NCJ):
                        nc.tensor.matmul(
                            pv_,
                            lhsT=xT[:, cj, ksl],
                            rhs=w_v_sb[:, cj, :],
                            start=(cj == 0),
                            stop=(cj == NCJ - 1),
                        )
                    for h in range(HPC):
                        nc.vector.tensor_copy(
                            out=v_nat[:, h, kb, 0:HD],
                            in_=pv_[:, h * HD:(h + 1) * HD],
                        )
                for w_sb, b_sb, dst in (
                    (w_k_sb, b_k_sb, kT),
                    (w_q_sb, b_q_sb, qT),
                ):
                    for h in range(HPC):
                        pm = ps_m.tile([HD, 512], F32, tag="pm")
                        for cj in range(NCJ):
                            nc.tensor.matmul(
                                pm,
                                lhsT=w_sb[:, cj, h, :],
                                rhs=xT[:, cj, tsl],
                                start=(cj == 0),
                                stop=(cj == NCJ - 1),
                            )
                        nc.vector.tensor_scalar_add(
                            out=dst[0:HD, h, tsl], in0=pm,
                            scalar1=b_sb[:, h:h + 1],
                        )

        # ---- phase 2/3: attention + projection, software-pipelined ----
        with (
            tc.tile_pool(name="oTp", bufs=1) as oTp,
            tc.tile_pool(name="expp", bufs=4) as expp,
            tc.tile_pool(name="recipp", bufs=2) as recipp,
            tc.tile_pool(name="bcsb", bufs=2) as bcsb,
            tc.tile_pool(name="ostage", bufs=3) as ostage,
            tc.tile_pool(name="ps_s", bufs=2, space="PSUM") as ps_s,
            tc.tile_pool(name="ps_o", bufs=2, space="PSUM") as ps_o,
            tc.tile_pool(name="ps_p", bufs=2, space="PSUM") as ps_p,
        ):
            # oT zero-padded to K=128 (rows 64:128 stay 0; w_o rows there are
            # host-zeroed) so the projection avoids the K=64/M=128 slow path
            oT = oTp.tile([128, HPC, N], BF16)
            nc.gpsimd.memset(oT[HD:128, :, :], 0.0)
            pending_recip = []  # flushed @g1 of the following unit (DVE)
            pending_bc = []     # flushed @g5 (PE bcast + DVE normalize)
            pending_proj = []   # flushed @g7 (PE matmuls)

            def make_tail(po, h, qsl, u):
                state = {}

                def recip():
                    # rows 1:128 zeroed so the K=128 broadcast matmul below
                    # multiplies them by e0's zeros without NaN risk
                    rt = recipp.tile([128, 512], F32, name="rt")
                    nc.vector.memset(rt, 0.0)
                    nc.vector.reciprocal(out=rt[0:1, :], in_=po[HD:HD + 1, :])
                    state["rt"] = rt

                def bcmult():
                    # broadcast across 64 partitions: e0[128,64].T @ rt[128,512]
                    pbc = ps_p.tile([128, C], F32, name="pp")
                    nc.tensor.matmul(
                        pbc[0:HD, :], lhsT=e0_mat, rhs=state["rt"],
                        start=True, stop=True,
                    )
                    bc = bcsb.tile([HD, 512], F32, name="bc")
                    nc.vector.tensor_copy(out=bc, in_=pbc[0:HD, :])
                    nc.vector.tensor_mul(
                        out=oT[0:HD, h, qsl], in0=po[0:HD, :], in1=bc
                    )
                return recip, bcmult

            def make_proj(qb, j):
                def proj():
                    q0 = qb * 512 + j * 128
                    pp = ps_p.tile([128, C], F32, name="pp")
                    for h in range(HPC):
                        nc.tensor.matmul(
                            pp,
                            lhsT=oT[:, h, q0:q0 + 128],
                            rhs=w_o_sb[:, h, :],
                            start=(h == 0),
                            stop=(h == HPC - 1),
                        )
                    ot = ostage.tile([128, C], F32, name="ot")
                    nc.vector.tensor_copy(out=ot, in_=pp)
                    nc.sync.dma_start(out=out[q0:q0 + 128, :], in_=ot)
                return proj

            units = [(qb, h) for qb in range(QB) for h in range(HPC)]

            def s_group(u, g):
                qb, h = units[u]
                qsl = slice(qb * 512, (qb + 1) * 512)
                ks = ps_s.tile([128, GP, 512], F32, name="ks")
                for j in range(GP):
                    kb = g * GP + j
                    nc.tensor.matmul(
                        ks[:, j, :],
                        lhsT=kT[:, h, kb * 128:(kb + 1) * 128],
                        rhs=qT[:, h, qsl],
                        start=True,
                        stop=True,
                    )
                return ks

            # flat (unit, group) pipeline: the scores skew carries across
            # unit boundaries so the PE/ACT streams never drain
            flat = [(u, g) for u in range(len(units)) for g in range(NG)]
            po_tiles = {}
            pend = s_group(*flat[0])
            for i, (u, g) in enumerate(flat):
                ks = pend
                pend = s_group(*flat[i + 1]) if i + 1 < len(flat) else None
                if g == 1:
                    for f in pending_recip:
                        f()
                    pending_recip.clear()
                elif g == 8:
                    for f in pending_bc:
                        f()
                    pending_bc.clear()
                elif g in (9, 11, 13, 15) and pending_proj:
                    pending_proj.pop(0)()
                et = expp.tile([128, GP, 512], BF16)
                nc.scalar.activation(out=et, in_=ks, func=AF.Exp, scale=SCALE)
                qb, h = units[u]
                if g == 0:
                    po_tiles[u] = ps_o.tile([HD + 1, 512], F32, name="po")
                po = po_tiles[u]
                for j in range(GP):
                    kb = g * GP + j
                    nc.tensor.matmul(
                        po,
                        lhsT=v_nat[:, h, kb, 0:HD + 1],
                        rhs=et[:, j, :],
                        start=(kb == 0),
                        stop=(kb == NT - 1),
                    )
                if g == NG - 1:
                    qsl = slice(qb * 512, (qb + 1) * 512)
                    recip, bcmult = make_tail(po_tiles.pop(u), h, qsl, u)
                    pending_recip.append(recip)
                    pending_bc.append(bcmult)
                    if h == HPC - 1:
                        for j in range(4):
                            pending_proj.append(make_proj(qb, j))
            for f in pending_recip:
                f()
            for f in pending_bc:
                f()
            for f in pending_proj:
                f()

    return nc


_PROGRAM = None


def _get_program():
    global _PROGRAM
    if _PROGRAM is None:
        _PROGRAM = _build_program()
    return _PROGRAM


def _bf16(a):
    import ml_dtypes

    return np.asarray(a, dtype=np.float32).astype(ml_dtypes.bfloat16)


def _prep_core_inputs(x, W_qkv, b_qkv, heads, batch):
    """Host-side slicing/relayout for one core."""
    cols = np.concatenate([np.arange(h * HD, (h + 1) * HD) for h in heads])
    w_q = W_qkv[:, cols]               # [512, 128]
    w_k = W_qkv[:, C + cols]
    w_v = W_qkv[:, 2 * C + cols]
    # [512, 128] -> [128 partitions, NCJ chunks, ...]
    w_q = np.ascontiguousarray(
        w_q.reshape(NCJ, 128, HPC, HD).transpose(1, 0, 2, 3))
    w_k = np.ascontiguousarray(
        w_k.reshape(NCJ, 128, HPC, HD).transpose(1, 0, 2, 3))
    w_v = np.ascontiguousarray(
        w_v.reshape(NCJ, 128, HPC * HD).transpose(1, 0, 2))
    b_q = np.ascontiguousarray(b_qkv[cols].reshape(HPC, HD).T)      # [64, 2]
    b_k = np.ascontiguousarray(b_qkv[C + cols].reshape(HPC, HD).T)
    xt = np.ascontiguousarray(
        x[batch].T.reshape(NCJ, 128, N).transpose(1, 0, 2))
    return {
        "xt": _bf16(xt),
        "w_q": _bf16(w_q),
        "w_k": _bf16(w_k),
        "w_v": _bf16(w_v),
        "b_q": b_q.astype(np.float32),
        "b_k": b_k.astype(np.float32),
    }


def _core_w_o(W_out, heads):
    rows = np.concatenate([np.arange(h * HD, (h + 1) * HD) for h in heads])
    w = np.zeros((128, HPC, C), dtype=np.float32)
    w[0:HD] = W_out[rows].reshape(HPC, HD, C).transpose(1, 0, 2)
    return _bf16(w)


def kernel(x, W_qkv, b_qkv, W_out, b_out):
    x = np.asarray(x, dtype=np.float32)
    W_qkv = np.asarray(W_qkv, dtype=np.float32)
    b_qkv = np.asarray(b_qkv, dtype=np.float32)
    W_out = np.asarray(W_out, dtype=np.float32)
    b_out = np.asarray(b_out, dtype=np.float32)

    nc = _get_program()
    in_maps = []
    for c in range(NCORES):
        batch, hp = c // 4, c % 4
        heads = [2 * hp, 2 * hp + 1]
        im = _prep_core_inputs(x, W_qkv, b_qkv, heads, batch)
        im["w_o"] = _core_w_o(W_out, heads)
        in_maps.append(im)

    res = run_bass_kernel_spmd(nc, in_maps, core_ids=list(range(NCORES)))

    # v-bias commutes: softmax rows sum to 1, so (P @ (V + 1 b_v)) @ W_o
    # = P@V@W_o + b_v@W_o. Add b_v@W_out and b_out once on the host.
    const_row = b_qkv[2 * C:] @ W_out + b_out    # [512]
    out = np.empty((B, N, C), dtype=np.float32)
    for b in range(B):
        acc = res.results[4 * b]["out"].astype(np.float32).copy()
        for c in range(4 * b + 1, 4 * b + 4):
            acc += res.results[c]["out"]
        out[b] = acc + const_row
    return out


# revision 49
# speedup vs baseline: 1.3266x; 1.1036x over previous
"""Multi-head attention (B=2, N=4096, C=512, H=8) on 8 TRN2 NeuronCores.

Sharding: core c handles batch c//4 and heads {2*(c%4), 2*(c%4)+1}
(data parallel over batch, tensor parallel over heads). Each core
computes its 2 heads' attention plus a partial output projection;
the host sums the 4 partials per batch and adds the bias terms
(b_out and b_v @ W_out, which commutes past softmax-weighted sums).

Compute layout per core (matmul operands bf16, accumulation f32):
  xT    = x.T via 2-byte DMA transpose (x pre-cast bf16 on host)
  qT,kT = per-head rows of (x @ Wq + bq).T etc.    [64, 2, N] bf16
  v     = x @ Wv, DMA-transposed to natural layout with a ones column
          per head (ones turns P@[V|1] into [P@V | rowsum(P)])
  per (512-query block, head), groups of 2 key-chunks:
     scoresT = kT_kb.T(stationary) @ qT   -> PSUM [128, 2, 512] f32
     expT    = Exp(scoresT / sqrt(C))     -> SBUF bf16 (one wide ACT op)
     accum P.T @ [v|1] over kb            -> PSUM [65, 512] f32
  tail (deferred one unit so the PE stream never blocks): reciprocal of
  row 64, partition-broadcast via a DRAM round-trip, normalize+cast bf16
  proj (deferred two units): oT.T @ W_out rows (K=64 per head, accum)
"""

import numpy as np

import concourse.bass as bass
import concourse.mybir as mybir
import concourse.tile as tile
from concourse.bass_utils import run_bass_kernel_spmd
from concourse.tile_rust import add_dep_helper
from concourse.vector_clock import ScopedClock

F32 = mybir.dt.float32
BF16 = mybir.dt.bfloat16
AF = mybir.ActivationFunctionType

B, N, C, H = 2, 4096, 512, 8
HD = C // H          # 64
HPC = H // 4         # 2 heads per core
NCORES = 8
NT = N // 128        # 32 key chunks
NCJ = C // 128       # 4 contraction chunks
QB = N // 512        # 8 query blocks
GP = 2               # key chunks per exp group (PSUM banks per scores tile)
NG = NT // GP
NUNITS = QB * HPC
SCALE = 1.0 / float(np.sqrt(C))


def _patch_tail_drain():
    """This walrus build caps sync waits at 1 per non-EventSemaphore
    instruction (2 for EventSemaphore); the stock TileContext tail-drain
    attaches every outstanding wait to one Drain, and the scheduler can
    leave >1 wait on regular instructions. Spill extras onto fresh
    same-engine nops inserted just before the over-subscribed one."""
    if getattr(tile.TileContext, "_drain_patched", False):
        return

    def _spill_excess_waits(nc):
        for fn in nc.m.functions:
            for bb in fn.blocks:
                insts = bb.instructions
                i = 0
                while i < len(insts):
                    inst = insts[i]
                    si = inst.sync_info
                    cap = 2 if isinstance(inst, mybir.InstEventSemaphore) else 1
                    if si is None or len(si.on_wait) <= cap:
                        i += 1
                        continue
                    extra = list(si.on_wait[cap:])
                    si.on_wait[:] = si.on_wait[:cap]
                    for w in extra:
                        nop = nc.engines[inst.engine].nop(
                            hint="wait_spill", nofuse=True
                        )
                        cur = nc.cur_bb.bb.instructions
                        cur.remove(nop.ins)
                        if nop.ins.sync_info is None:
                            nop.ins.sync_info = mybir.SyncInfo(
                                on_update=[], on_wait=[]
                            )
                        nop.ins.sync_info.on_wait.append(w)
                        insts.insert(i, nop.ins)
                        i += 1
                    i += 1

    def _drain_and_barrier(self, tick_clock, wait_clock):
        nc = self.nc
        drain_inst = nc.sync.drain()
        wait_clock.add_sem_waits(
            drain_inst.ins, ScopedClock({None: tick_clock.global_clock})
        )
        nc.all_engine_barrier()
        assert self.sems is not None
        popped = nc._tile_sem_poison_stack.pop()
        assert popped is self._sem_poison
        nc.clear_and_free_semaphores(list(self.sems.allocated().values()))
        nc.all_engine_barrier()
        _spill_excess_waits(nc)

    tile.TileContext._drain_and_barrier = _drain_and_barrier
    tile.TileContext._drain_patched = True


def _build_program():
    _patch_tail_drain()
    nc = bass.Bass()

    xt = nc.dram_tensor("xt", [128, NCJ, N], BF16, kind="ExternalInput")
    # host-prearranged weight layouts (see kernel() below)
    w_q = nc.dram_tensor("w_q", [128, NCJ, HPC, HD], BF16, kind="ExternalInput")
    w_k = nc.dram_tensor("w_k", [128, NCJ, HPC, HD], BF16, kind="ExternalInput")
    w_v = nc.dram_tensor("w_v", [128, NCJ, HPC * HD], BF16, kind="ExternalInput")
    w_o = nc.dram_tensor("w_o", [128, HPC, C], BF16, kind="ExternalInput")
    b_q = nc.dram_tensor("b_q", [HD, HPC], F32, kind="ExternalInput")
    b_k = nc.dram_tensor("b_k", [HD, HPC], F32, kind="ExternalInput")
    out = nc.dram_tensor("out", [N, C], F32, kind="ExternalOutput")

    from contextlib import ExitStack

    with tile.TileContext(nc) as tc, ExitStack() as ctx:
        const = ctx.enter_context(tc.tile_pool(name="const", bufs=1))
        w_q_sb = const.tile([128, NCJ, HPC, HD], BF16)
        w_k_sb = const.tile([128, NCJ, HPC, HD], BF16)
        w_v_sb = const.tile([128, NCJ, HPC * HD], BF16)
        w_o_sb = const.tile([128, HPC, C], BF16)
        b_q_sb = const.tile([HD, HPC], F32)
        b_k_sb = const.tile([HD, HPC], F32)
        # e0 selector: column m of lhsT is e0 -> out[m, :] = rhs row 0
        # (K=128 broadcast matmul; K=1/K=64-with-M=128 shapes run half-rate)
        e0_mat = const.tile([128, HD], F32)
        nc.vector.memset(e0_mat, 0.0)
        nc.vector.memset(e0_mat[0:1, :], 1.0)
        nc.gpsimd.dma_start(out=w_q_sb, in_=w_q[:])
        nc.gpsimd.dma_start(out=w_k_sb, in_=w_k[:])
        nc.gpsimd.dma_start(out=w_v_sb, in_=w_v[:])
        nc.gpsimd.dma_start(out=w_o_sb, in_=w_o[:])
        nc.gpsimd.dma_start(out=b_q_sb, in_=b_q[:])
        nc.gpsimd.dma_start(out=b_k_sb, in_=b_k[:])

        persist = ctx.enter_context(tc.tile_pool(name="persist", bufs=1))
        # K=128 zero-padded: rows 0:64 hold the head's q/k rows, 64:128 stay
        # zero (K=64 M=128 single-group matmuls run at half rate on this HW)
        qT = persist.tile([128, HPC, N], BF16)
        kT = persist.tile([128, HPC, N], BF16)
        nc.gpsimd.memset(qT[HD:128, :, :], 0.0)
        nc.gpsimd.memset(kT[HD:128, :, :], 0.0)
        # [tokens, head, kb, 128]: dims at 0:64, ones at 64, rest padding --
        # each (h, kb) block starts 256B-aligned for the xbar transpose DMA
        v_nat = persist.tile([128, HPC, NT, 128], BF16)

        # ---- phase 0/1: load host-transposed xT, qkv projections ----
        with (
            tc.tile_pool(name="xTp", bufs=1) as xTp,
            tc.tile_pool(name="ps_m", bufs=3, space="PSUM") as ps_m,
            tc.tile_pool(name="ps_v", bufs=3, space="PSUM") as ps_v,
        ):
            xT = xTp.tile([128, NCJ, N], BF16)
            for tb in range(QB):
                tsl = slice(tb * 512, (tb + 1) * 512)
                nc.sync.dma_start(out=xT[:, :, tsl], in_=xt[:, :, tsl])
            nc.vector.memset(v_nat, 1.0)
            for tb in range(QB):
                tsl = slice(tb * 512, (tb + 1) * 512)
                # v first, projected straight into natural layout
                # (lhsT = xT token block, rhs = W_v), no transposes anywhere
                for kb in range(tb * 4, tb * 4 + 4):
                    ksl = slice(kb * 128, (kb + 1) * 128)
                    pv_ = ps_v.tile([128, HPC * HD], F32, name="pv_")
                    for cj in range(NCJ):
                        nc.tensor.matmul(
                            pv_,
                            lhsT=xT[:, cj, ksl],
                            rhs=w_v_sb[:, cj, :],
                            start=(cj == 0),
                            stop=(cj == NCJ - 1),
                        )
                    for h in range(HPC):
                        nc.vector.tensor_copy(
                            out=v_nat[:, h, kb, 0:HD],
                            in_=pv_[:, h * HD:(h + 1) * HD],
                        )
                for w_sb, b_sb, dst in (
                    (w_k_sb, b_k_sb, kT),
                    (w_q_sb, b_q_sb, qT),
                ):
                    for h in range(HPC):
                        pm = ps_m.tile([HD, 512], F32, tag="pm")
                        for cj in range(NCJ):
                            nc.tensor.matmul(
                                pm,
                                lhsT=w_sb[:, cj, h, :],
                                rhs=xT[:, cj, tsl],
                                start=(cj == 0),
                                stop=(cj == NCJ - 1),
                            )
                        nc.vector.tensor_scalar_add(
                            out=dst[0:HD, h, tsl], in0=pm,
                            scalar1=b_sb[:, h:h + 1],
                        )

        # ---- phase 2/3: attention + projection, software-pipelined ----
        with (
            tc.tile_pool(name="oTp", bufs=1) as oTp,
            tc.tile_pool(name="expp", bufs=4) as expp,
            tc.tile_pool(name="recipp", bufs=2) as recipp,
            tc.tile_pool(name="bcsb", bufs=2) as bcsb,
            tc.tile_pool(name="ostage", bufs=3) as ostage,
            tc.tile_pool(name="ps_s", bufs=2, space="PSUM") as ps_s,
            tc.tile_pool(name="ps_o", bufs=2, space="PSUM") as ps_o,
            tc.tile_pool(name="ps_p", bufs=2, space="PSUM") as ps_p,
        ):
            # oT zero-padded to K=128 (rows 64:128 stay 0; w_o rows there are
            # host-zeroed) so the projection avoids the K=64/M=128 slow path
            oT = oTp.tile([128, HPC, N], BF16)
            nc.gpsimd.memset(oT[HD:128, :, :], 0.0)
            last_exp = {"inst": None}
            pending_recip = []  # flushed @g1 of the following unit (DVE)
            pending_bc = []     # flushed @g5 (PE bcast + DVE normalize)
            pending_proj = []   # flushed @g7 (PE matmuls)

            def make_tail(po, h, qsl, u):
                state = {}

                def recip():
                    # rows 1:128 zeroed so the K=128 broadcast matmul below
                    # multiplies them by e0's zeros without NaN risk
                    rt = recipp.tile([128, 512], F32, name="rt")
                    nc.vector.memset(rt, 0.0)
                    nc.vector.reciprocal(out=rt[0:1, :], in_=po[HD:HD + 1, :])
                    state["rt"] = rt

                def bcmult():
                    # broadcast across 64 partitions: e0[128,64].T @ rt[128,512]
                    pbc = ps_p.tile([128, C], F32, name="pp")
                    mm = nc.tensor.matmul(
                        pbc[0:HD, :], lhsT=e0_mat, rhs=state["rt"],
                        start=True, stop=True,
                    )
                    if last_exp["inst"] is not None:
                        # keep this off the PE stream until the reciprocal
                        # (6x slower than the scheduler's model) has finished
                        add_dep_helper(
                            mm.ins, last_exp["inst"], sync=False,
                            reason="bc after recip really done",
                        )
                    bc = bcsb.tile([HD, 512], F32, name="bc")
                    nc.vector.tensor_copy(out=bc, in_=pbc[0:HD, :])
                    nc.vector.tensor_mul(
                        out=oT[0:HD, h, qsl], in0=po[0:HD, :], in1=bc
                    )
                return recip, bcmult

            def make_proj(qb, j):
                def proj():
                    q0 = qb * 512 + j * 128
                    pp = ps_p.tile([128, C], F32, name="pp")
                    for h in range(HPC):
                        mm = nc.tensor.matmul(
                            pp,
                            lhsT=oT[:, h, q0:q0 + 128],
                            rhs=w_o_sb[:, h, :],
                            start=(h == 0),
                            stop=(h == HPC - 1),
                        )
                        if h == 0 and last_exp["inst"] is not None:
                            add_dep_helper(
                                mm.ins, last_exp["inst"], sync=False,
                                reason="proj after normalize really done",
                            )
                    ot = ostage.tile([128, C], F32, name="ot")
                    nc.vector.tensor_copy(out=ot, in_=pp)
                    nc.sync.dma_start(out=out[q0:q0 + 128, :], in_=ot)
                return proj

            units = [(qb, h) for qb in range(QB) for h in range(HPC)]

            def s_group(u, g):
                qb, h = units[u]
                qsl = slice(qb * 512, (qb + 1) * 512)
                ks = ps_s.tile([128, GP, 512], F32, name="ks")
                for j in range(GP):
                    kb = g * GP + j
                    nc.tensor.matmul(
                        ks[:, j, :],
                        lhsT=kT[:, h, kb * 128:(kb + 1) * 128],
                        rhs=qT[:, h, qsl],
                        start=True,
                        stop=True,
                    )
                return ks

            # flat (unit, group) pipeline: the scores skew carries across
            # unit boundaries so the PE/ACT streams never drain
            flat = [(u, g) for u in range(len(units)) for g in range(NG)]
            po_tiles = {}
            pend = s_group(*flat[0])
            for i, (u, g) in enumerate(flat):
                ks = pend
                pend = s_group(*flat[i + 1]) if i + 1 < len(flat) else None
                if g == 1:
                    for f in pending_recip:
                        f()
                    pending_recip.clear()
                elif g == 8:
                    for f in pending_bc:
                        f()
                    pending_bc.clear()
                elif g in (9, 11, 13, 15) and pending_proj:
                    pending_proj.pop(0)()
                et = expp.tile([128, GP, 512], BF16)
                exp_bi = nc.scalar.activation(
                    out=et, in_=ks, func=AF.Exp, scale=SCALE
                )
                last_exp["inst"] = exp_bi.ins
                qb, h = units[u]
                if g == 0:
                    po_tiles[u] = ps_o.tile([HD + 1, 512], F32, name="po")
                po = po_tiles[u]
                for j in range(GP):
                    kb = g * GP + j
                    nc.tensor.matmul(
                        po,
                        lhsT=v_nat[:, h, kb, 0:HD + 1],
                        rhs=et[:, j, :],
                        start=(kb == 0),
                        stop=(kb == NT - 1),
                    )
                if g == NG - 1:
                    qsl = slice(qb * 512, (qb + 1) * 512)
                    recip, bcmult = make_tail(po_tiles.pop(u), h, qsl, u)
                    pending_recip.append(recip)
                    pending_bc.append(bcmult)
                    if h == HPC - 1:
                        for j in range(4):
                            pending_proj.append(make_proj(qb, j))
            for f in pending_recip:
                f()
            for f in pending_bc:
                f()
            for f in pending_proj:
                f()

    return nc


_PROGRAM = None


def _get_program():
    global _PROGRAM
    if _PROGRAM is None:
        _PROGRAM = _build_program()
    return _PROGRAM


def _bf16(a):
    import ml_dtypes

    return np.asarray(a, dtype=np.float32).astype(ml_dtypes.bfloat16)


def _prep_core_inputs(x, W_qkv, b_qkv, heads, batch):
    """Host-side slicing/relayout for one core."""
    cols = np.concatenate([np.arange(h * HD, (h + 1) * HD) for h in heads])
    w_q = W_qkv[:, cols]               # [512, 128]
    w_k = W_qkv[:, C + cols]
    w_v = W_qkv[:, 2 * C + cols]
    # [512, 128] -> [128 partitions, NCJ chunks, ...]
    w_q = np.ascontiguousarray(
        w_q.reshape(NCJ, 128, HPC, HD).transpose(1, 0, 2, 3))
    w_k = np.ascontiguousarray(
        w_k.reshape(NCJ, 128, HPC, HD).transpose(1, 0, 2, 3))
    w_v = np.ascontiguousarray(
        w_v.reshape(NCJ, 128, HPC * HD).transpose(1, 0, 2))
    b_q = np.ascontiguousarray(b_qkv[cols].reshape(HPC, HD).T)      # [64, 2]
    b_k = np.ascontiguousarray(b_qkv[C + cols].reshape(HPC, HD).T)
    xt = np.ascontiguousarray(
        x[batch].T.reshape(NCJ, 128, N).transpose(1, 0, 2))
    return {
        "xt": _bf16(xt),
        "w_q": _bf16(w_q),
        "w_k": _bf16(w_k),
        "w_v": _bf16(w_v),
        "b_q": b_q.astype(np.float32),
        "b_k": b_k.astype(np.float32),
    }


def _core_w_o(W_out, heads):
    rows = np.concatenate([np.arange(h * HD, (h + 1) * HD) for h in heads])
    w = np.zeros((128, HPC, C), dtype=np.float32)
    w[0:HD] = W_out[rows].reshape(HPC, HD, C).transpose(1, 0, 2)
    return _bf16(w)


def kernel(x, W_qkv, b_qkv, W_out, b_out):
    x = np.asarray(x, dtype=np.float32)
    W_qkv = np.asarray(W_qkv, dtype=np.float32)
    b_qkv = np.asarray(b_qkv, dtype=np.float32)
    W_out = np.asarray(W_out, dtype=np.float32)
    b_out = np.asarray(b_out, dtype=np.float32)

    nc = _get_program()
    in_maps = []
    for c in range(NCORES):
        batch, hp = c // 4, c % 4
        heads = [2 * hp, 2 * hp + 1]
        im = _prep_core_inputs(x, W_qkv, b_qkv, heads, batch)
        im["w_o"] = _core_w_o(W_out, heads)
        in_maps.append(im)

    res = run_bass_kernel_spmd(nc, in_maps, core_ids=list(range(NCORES)))

    # v-bias commutes: softmax rows sum to 1, so (P @ (V + 1 b_v)) @ W_o
    # = P@V@W_o + b_v@W_o. Add b_v@W_out and b_out once on the host.
    const_row = b_qkv[2 * C:] @ W_out + b_out    # [512]
    out = np.empty((B, N, C), dtype=np.float32)
    for b in range(B):
        acc = res.results[4 * b]["out"].astype(np.float32).copy()
        for c in range(4 * b + 1, 4 * b + 4):
            acc += res.results[c]["out"]
        out[b] = acc + const_row
    return out


# revision 52
# speedup vs baseline: 1.3366x; 1.0075x over previous
"""Multi-head attention (B=2, N=4096, C=512, H=8) on 8 TRN2 NeuronCores.

Sharding: core c handles batch c//4 and heads {2*(c%4), 2*(c%4)+1}
(data parallel over batch, tensor parallel over heads). Each core
computes its 2 heads' attention plus a partial output projection;
the host sums the 4 partials per batch and adds the bias terms
(b_out and b_v @ W_out, which commutes past softmax-weighted sums).

Compute layout per core (matmul operands bf16, accumulation f32):
  xT    = x.T via 2-byte DMA transpose (x pre-cast bf16 on host)
  qT,kT = per-head rows of (x @ Wq + bq).T etc.    [64, 2, N] bf16
  v     = x @ Wv, DMA-transposed to natural layout with a ones column
          per head (ones turns P@[V|1] into [P@V | rowsum(P)])
  per (512-query block, head), groups of 2 key-chunks:
     scoresT = kT_kb.T(stationary) @ qT   -> PSUM [128, 2, 512] f32
     expT    = Exp(scoresT / sqrt(C))     -> SBUF bf16 (one wide ACT op)
     accum P.T @ [v|1] over kb            -> PSUM [65, 512] f32
  tail (deferred one unit so the PE stream never blocks): reciprocal of
  row 64, partition-broadcast via a DRAM round-trip, normalize+cast bf16
  proj (deferred two units): oT.T @ W_out rows (K=64 per head, accum)
"""

import numpy as np

import concourse.bass as bass
import concourse.mybir as mybir
import concourse.tile as tile
from concourse.bass_utils import run_bass_kernel_spmd
from concourse.tile_rust import add_dep_helper
from concourse.vector_clock import ScopedClock

F32 = mybir.dt.float32
BF16 = mybir.dt.bfloat16
AF = mybir.ActivationFunctionType

B, N, C, H = 2, 4096, 512, 8
HD = C // H          # 64
HPC = H // 4         # 2 heads per core
NCORES = 8
NT = N // 128        # 32 key chunks
NCJ = C // 128       # 4 contraction chunks
QB = N // 512        # 8 query blocks
GP = 2               # key chunks per exp group (PSUM banks per scores tile)
NG = NT // GP
NUNITS = QB * HPC
SCALE = 1.0 / float(np.sqrt(C))


def _patch_tail_drain():
    """This walrus build caps sync waits at 1 per non-EventSemaphore
    instruction (2 for EventSemaphore); the stock TileContext tail-drain
    attaches every outstanding wait to one Drain, and the scheduler can
    leave >1 wait on regular instructions. Spill extras onto fresh
    same-engine nops inserted just before the over-subscribed one."""
    if getattr(tile.TileContext, "_drain_patched", False):
        return

    def _spill_excess_waits(nc):
        for fn in nc.m.functions:
            for bb in fn.blocks:
                insts = bb.instructions
                i = 0
                while i < len(insts):
                    inst = insts[i]
                    si = inst.sync_info
                    cap = 2 if isinstance(inst, mybir.InstEventSemaphore) else 1
                    if si is None or len(si.on_wait) <= cap:
                        i += 1
                        continue
                    extra = list(si.on_wait[cap:])
                    si.on_wait[:] = si.on_wait[:cap]
                    for w in extra:
                        nop = nc.engines[inst.engine].nop(
                            hint="wait_spill", nofuse=True
                        )
                        cur = nc.cur_bb.bb.instructions
                        cur.remove(nop.ins)
                        if nop.ins.sync_info is None:
                            nop.ins.sync_info = mybir.SyncInfo(
                                on_update=[], on_wait=[]
                            )
                        nop.ins.sync_info.on_wait.append(w)
                        insts.insert(i, nop.ins)
                        i += 1
                    i += 1

    def _drain_and_barrier(self, tick_clock, wait_clock):
        nc = self.nc
        drain_inst = nc.sync.drain()
        wait_clock.add_sem_waits(
            drain_inst.ins, ScopedClock({None: tick_clock.global_clock})
        )
        nc.all_engine_barrier()
        assert self.sems is not None
        popped = nc._tile_sem_poison_stack.pop()
        assert popped is self._sem_poison
        nc.clear_and_free_semaphores(list(self.sems.allocated().values()))
        nc.all_engine_barrier()
        _spill_excess_waits(nc)

    tile.TileContext._drain_and_barrier = _drain_and_barrier
    tile.TileContext._drain_patched = True


def _build_program():
    _patch_tail_drain()
    nc = bass.Bass()

    xt = nc.dram_tensor("xt", [128, NCJ, N], BF16, kind="ExternalInput")
    # host-prearranged weight layouts (see kernel() below)
    w_q = nc.dram_tensor("w_q", [128, NCJ, HPC, HD], BF16, kind="ExternalInput")
    w_k = nc.dram_tensor("w_k", [128, NCJ, HPC, HD], BF16, kind="ExternalInput")
    w_v = nc.dram_tensor("w_v", [128, NCJ, HPC * HD], BF16, kind="ExternalInput")
    w_o = nc.dram_tensor("w_o", [128, HPC, C], BF16, kind="ExternalInput")
    b_q = nc.dram_tensor("b_q", [HD, HPC], F32, kind="ExternalInput")
    b_k = nc.dram_tensor("b_k", [HD, HPC], F32, kind="ExternalInput")
    out = nc.dram_tensor("out", [N, C], F32, kind="ExternalOutput")
    scratch = nc.dram_tensor("scratch", [NUNITS, 512], F32)

    from contextlib import ExitStack

    with tile.TileContext(nc) as tc, ExitStack() as ctx:
        const = ctx.enter_context(tc.tile_pool(name="const", bufs=1))
        w_q_sb = const.tile([128, NCJ, HPC, HD], BF16)
        w_k_sb = const.tile([128, NCJ, HPC, HD], BF16)
        w_v_sb = const.tile([128, NCJ, HPC * HD], BF16)
        w_o_sb = const.tile([128, HPC, C], BF16)
        b_q_sb = const.tile([HD, HPC], F32)
        b_k_sb = const.tile([HD, HPC], F32)

        nc.gpsimd.dma_start(out=w_q_sb, in_=w_q[:])
        nc.gpsimd.dma_start(out=w_k_sb, in_=w_k[:])
        nc.gpsimd.dma_start(out=w_v_sb, in_=w_v[:])
        nc.gpsimd.dma_start(out=w_o_sb, in_=w_o[:])
        nc.gpsimd.dma_start(out=b_q_sb, in_=b_q[:])
        nc.gpsimd.dma_start(out=b_k_sb, in_=b_k[:])

        persist = ctx.enter_context(tc.tile_pool(name="persist", bufs=1))
        # K=128 zero-padded: rows 0:64 hold the head's q/k rows, 64:128 stay
        # zero (K=64 M=128 single-group matmuls run at half rate on this HW)
        qT = persist.tile([128, HPC, N], BF16)
        kT = persist.tile([128, HPC, N], BF16)
        nc.gpsimd.memset(qT[HD:128, :, :], 0.0)
        nc.gpsimd.memset(kT[HD:128, :, :], 0.0)
        # [tokens, head, kb, 128]: dims at 0:64, ones at 64, rest padding --
        # each (h, kb) block starts 256B-aligned for the xbar transpose DMA
        v_nat = persist.tile([128, HPC, NT, 128], BF16)

        # ---- phase 0/1: load host-transposed xT, qkv projections ----
        with (
            tc.tile_pool(name="xTp", bufs=1) as xTp,
            tc.tile_pool(name="ps_m", bufs=3, space="PSUM") as ps_m,
            tc.tile_pool(name="ps_v", bufs=3, space="PSUM") as ps_v,
        ):
            xT = xTp.tile([128, NCJ, N], BF16)
            for tb in range(QB):
                tsl = slice(tb * 512, (tb + 1) * 512)
                nc.sync.dma_start(out=xT[:, :, tsl], in_=xt[:, :, tsl])
            nc.vector.memset(v_nat, 1.0)
            for tb in range(QB):
                tsl = slice(tb * 512, (tb + 1) * 512)
                # v first, projected straight into natural layout
                # (lhsT = xT token block, rhs = W_v), no transposes anywhere
                for kb in range(tb * 4, tb * 4 + 4):
                    ksl = slice(kb * 128, (kb + 1) * 128)
                    pv_ = ps_v.tile([128, HPC * HD], F32, name="pv_")
                    for cj in range(NCJ):
                        nc.tensor.matmul(
                            pv_,
                            lhsT=xT[:, cj, ksl],
                            rhs=w_v_sb[:, cj, :],
                            start=(cj == 0),
                            stop=(cj == NCJ - 1),
                        )
                    for h in range(HPC):
                        nc.vector.tensor_copy(
                            out=v_nat[:, h, kb, 0:HD],
                            in_=pv_[:, h * HD:(h + 1) * HD],
                        )
                for w_sb, b_sb, dst in (
                    (w_k_sb, b_k_sb, kT),
                    (w_q_sb, b_q_sb, qT),
                ):
                    for h in range(HPC):
                        pm = ps_m.tile([HD, 512], F32, tag="pm")
                        for cj in range(NCJ):
                            nc.tensor.matmul(
                                pm,
                                lhsT=w_sb[:, cj, h, :],
                                rhs=xT[:, cj, tsl],
                                start=(cj == 0),
                                stop=(cj == NCJ - 1),
                            )
                        nc.vector.tensor_scalar_add(
                            out=dst[0:HD, h, tsl], in0=pm,
                            scalar1=b_sb[:, h:h + 1],
                        )

        # ---- phase 2/3: attention + projection, software-pipelined ----
        with (
            tc.tile_pool(name="oTp", bufs=1) as oTp,
            tc.tile_pool(name="expp", bufs=4) as expp,
            tc.tile_pool(name="recipp", bufs=2) as recipp,
            tc.tile_pool(name="bcsb", bufs=2) as bcsb,
            tc.tile_pool(name="ostage", bufs=3) as ostage,
            tc.tile_pool(name="ps_s", bufs=2, space="PSUM") as ps_s,
            tc.tile_pool(name="ps_o", bufs=2, space="PSUM") as ps_o,
            tc.tile_pool(name="ps_p", bufs=2, space="PSUM") as ps_p,
        ):
            # oT zero-padded to K=128 (rows 64:128 stay 0; w_o rows there are
            # host-zeroed) so the projection avoids the K=64/M=128 slow path
            oT = oTp.tile([128, HPC, N], BF16)
            nc.gpsimd.memset(oT[HD:128, :, :], 0.0)
            last_exp = {"inst": None}
            pending_recip = []  # flushed @g1 of the following unit (DVE)
            pending_bc = []     # flushed @g5 (PE bcast + DVE normalize)
            pending_proj = []   # flushed @g7 (PE matmuls)

            def make_tail(po, h, qsl, u):
                state = {}

                def recip():
                    rt = recipp.tile([1, 512], F32, name="rt")
                    nc.vector.reciprocal(out=rt, in_=po[HD:HD + 1, :])
                    # round-trip through DRAM to broadcast across partitions
                    # (no on-chip partition-broadcast path); explicit dep
                    # edge orders the read-back after the write
                    state["wr"] = nc.sync.dma_start(
                        out=scratch[u:u + 1, :], in_=rt
                    )

                def bcmult():
                    bc = bcsb.tile([HD, 512], F32, name="bc")
                    rd = nc.gpsimd.dma_start(
                        out=bc,
                        in_=bass.AP(
                            tensor=scratch, offset=u * 512,
                            ap=[[0, HD], [1, 512]],
                        ),
                    )
                    add_dep_helper(
                        rd.ins, state["wr"].ins, sync=True,
                        reason="scratch broadcast RAW",
                    )
                    nc.vector.tensor_mul(
                        out=oT[0:HD, h, qsl], in0=po[0:HD, :], in1=bc
                    )
                return recip, bcmult

            def make_proj(qb, j):
                def proj():
                    q0 = qb * 512 + j * 128
                    pp = ps_p.tile([128, C], F32, name="pp")
                    for h in range(HPC):
                        mm = nc.tensor.matmul(
                            pp,
                            lhsT=oT[:, h, q0:q0 + 128],
                            rhs=w_o_sb[:, h, :],
                            start=(h == 0),
                            stop=(h == HPC - 1),
                        )
                        if h == 0 and last_exp["inst"] is not None:
                            add_dep_helper(
                                mm.ins, last_exp["inst"], sync=False,
                                reason="proj after normalize really done",
                            )
                    ot = ostage.tile([128, C], F32, name="ot")
                    nc.vector.tensor_copy(out=ot, in_=pp)
                    nc.sync.dma_start(out=out[q0:q0 + 128, :], in_=ot)
                return proj

            units = [(qb, h) for qb in range(QB) for h in range(HPC)]

            def s_group(u, g):
                qb, h = units[u]
                qsl = slice(qb * 512, (qb + 1) * 512)
                ks = ps_s.tile([128, GP, 512], F32, name="ks")
                for j in range(GP):
                    kb = g * GP + j
                    nc.tensor.matmul(
                        ks[:, j, :],
                        lhsT=kT[:, h, kb * 128:(kb + 1) * 128],
                        rhs=qT[:, h, qsl],
                        start=True,
                        stop=True,
                    )
                return ks

            # flat (unit, group) pipeline: the scores skew carries across
            # unit boundaries so the PE/ACT streams never drain
            flat = [(u, g) for u in range(len(units)) for g in range(NG)]
            po_tiles = {}
            pend = s_group(*flat[0])
            for i, (u, g) in enumerate(flat):
                ks = pend
                pend = s_group(*flat[i + 1]) if i + 1 < len(flat) else None
                if g == 1:
                    for f in pending_recip:
                        f()
                    pending_recip.clear()
                elif g == 8:
                    for f in pending_bc:
                        f()
                    pending_bc.clear()
                elif g in (9, 11, 13, 15) and pending_proj:
                    pending_proj.pop(0)()
                et = expp.tile([128, GP, 512], BF16)
                exp_bi = nc.scalar.activation(
                    out=et, in_=ks, func=AF.Exp, scale=SCALE
                )
                last_exp["inst"] = exp_bi.ins
                qb, h = units[u]
                if g == 0:
                    po_tiles[u] = ps_o.tile([HD + 1, 512], F32, name="po")
                po = po_tiles[u]
                for j in range(GP):
                    kb = g * GP + j
                    nc.tensor.matmul(
                        po,
                        lhsT=v_nat[:, h, kb, 0:HD + 1],
                        rhs=et[:, j, :],
                        start=(kb == 0),
                        stop=(kb == NT - 1),
                    )
                if g == NG - 1:
                    qsl = slice(qb * 512, (qb + 1) * 512)
                    recip, bcmult = make_tail(po_tiles.pop(u), h, qsl, u)
                    pending_recip.append(recip)
                    pending_bc.append(bcmult)
                    if h == HPC - 1:
                        for j in range(4):
                            pending_proj.append(make_proj(qb, j))
            for f in pending_recip:
                f()
            for f in pending_bc:
                f()
            for f in pending_proj:
                f()

    return nc


_PROGRAM = None


def _get_program():
    global _PROGRAM
    if _PROGRAM is None:
        _PROGRAM = _build_program()
    return _PROGRAM


def _bf16(a):
    import ml_dtypes

    return np.asarray(a, dtype=np.float32).astype(ml_dtypes.bfloat16)


def _prep_core_inputs(x, W_qkv, b_qkv, heads, batch):
    """Host-side slicing/relayout for one core."""
    cols = np.concatenate([np.arange(h * HD, (h + 1) * HD) for h in heads])
    w_q = W_qkv[:, cols]               # [512, 128]
    w_k = W_qkv[:, C + cols]
    w_v = W_qkv[:, 2 * C + cols]
    # [512, 128] -> [128 partitions, NCJ chunks, ...]
    w_q = np.ascontiguousarray(
        w_q.reshape(NCJ, 128, HPC, HD).transpose(1, 0, 2, 3))
    w_k = np.ascontiguousarray(
        w_k.reshape(NCJ, 128, HPC, HD).transpose(1, 0, 2, 3))
    w_v = np.ascontiguousarray(
        w_v.reshape(NCJ, 128, HPC * HD).transpose(1, 0, 2))
    b_q = np.ascontiguousarray(b_qkv[cols].reshape(HPC, HD).T)      # [64, 2]
    b_k = np.ascontiguousarray(b_qkv[C + cols].reshape(HPC, HD).T)
    xt = np.ascontiguousarray(
        x[batch].T.reshape(NCJ, 128, N).transpose(1, 0, 2))
    return {
        "xt": _bf16(xt),
        "w_q": _bf16(w_q),
        "w_k": _bf16(w_k),
        "w_v": _bf16(w_v),
        "b_q": b_q.astype(np.float32),
        "b_k": b_k.astype(np.float32),
    }


def _core_w_o(W_out, heads):
    rows = np.concatenate([np.arange(h * HD, (h + 1) * HD) for h in heads])
    w = np.zeros((128, HPC, C), dtype=np.float32)
    w[0:HD] = W_out[rows].reshape(HPC, HD, C).transpose(1, 0, 2)
    return _bf16(w)


def kernel(x, W_qkv, b_qkv, W_out, b_out):
    x = np.asarray(x, dtype=np.float32)
    W_qkv = np.asarray(W_qkv, dtype=np.float32)
    b_qkv = np.asarray(b_qkv, dtype=np.float32)
    W_out = np.asarray(W_out, dtype=np.float32)
    b_out = np.asarray(b_out, dtype=np.float32)

    nc = _get_program()
    in_maps = []
    for c in range(NCORES):
        batch, hp = c // 4, c % 4
        heads = [2 * hp, 2 * hp + 1]
        im = _prep_core_inputs(x, W_qkv, b_qkv, heads, batch)
        im["w_o"] = _core_w_o(W_out, heads)
        in_maps.append(im)

    res = run_bass_kernel_spmd(nc, in_maps, core_ids=list(range(NCORES)))

    # v-bias commutes: softmax rows sum to 1, so (P @ (V + 1 b_v)) @ W_o
    # = P@V@W_o + b_v@W_o. Add b_v@W_out and b_out once on the host.
    const_row = b_qkv[2 * C:] @ W_out + b_out    # [512]
    out = np.empty((B, N, C), dtype=np.float32)
    for b in range(B):
        acc = res.results[4 * b]["out"].astype(np.float32).copy()
        for c in range(4 * b + 1, 4 * b + 4):
            acc += res.results[c]["out"]
        out[b] = acc + const_row
    return out


# revision 57
# speedup vs baseline: 1.3726x; 1.0269x over previous
"""Multi-head attention (B=2, N=4096, C=512, H=8) on 8 TRN2 NeuronCores.

Sharding: core c handles batch c//4 and heads {2*(c%4), 2*(c%4)+1}
(data parallel over batch, tensor parallel over heads). Each core
computes its 2 heads' attention plus a partial output projection;
the host sums the 4 partials per batch and adds the bias terms
(b_out and b_v @ W_out, which commutes past softmax-weighted sums).

Compute layout per core (matmul operands bf16, accumulation f32):
  xT    = x.T via 2-byte DMA transpose (x pre-cast bf16 on host)
  qT,kT = per-head rows of (x @ Wq + bq).T etc.    [64, 2, N] bf16
  v     = x @ Wv, DMA-transposed to natural layout with a ones column
          per head (ones turns P@[V|1] into [P@V | rowsum(P)])
  per (512-query block, head), groups of 2 key-chunks:
     scoresT = kT_kb.T(stationary) @ qT   -> PSUM [128, 2, 512] f32
     expT    = Exp(scoresT / sqrt(C))     -> SBUF bf16 (one wide ACT op)
     accum P.T @ [v|1] over kb            -> PSUM [65, 512] f32
  tail (deferred one unit so the PE stream never blocks): reciprocal of
  row 64, partition-broadcast via a DRAM round-trip, normalize+cast bf16
  proj (deferred two units): oT.T @ W_out rows (K=64 per head, accum)
"""

import numpy as np

import concourse.bass as bass
import concourse.mybir as mybir
import concourse.tile as tile
from concourse.bass_utils import run_bass_kernel_spmd
from concourse.tile_rust import add_dep_helper
from concourse.vector_clock import ScopedClock

F32 = mybir.dt.float32
BF16 = mybir.dt.bfloat16
AF = mybir.ActivationFunctionType

B, N, C, H = 2, 4096, 512, 8
HD = C // H          # 64
HPC = H // 4         # 2 heads per core
NCORES = 8
NT = N // 128        # 32 key chunks
NCJ = C // 128       # 4 contraction chunks
QB = N // 512        # 8 query blocks
GP = 2               # key chunks per exp group (PSUM banks per scores tile)
NG = NT // GP
NUNITS = QB * HPC
SCALE = 1.0 / float(np.sqrt(C))


def _patch_tail_drain():
    """This walrus build caps sync waits at 1 per non-EventSemaphore
    instruction (2 for EventSemaphore); the stock TileContext tail-drain
    attaches every outstanding wait to one Drain, and the scheduler can
    leave >1 wait on regular instructions. Spill extras onto fresh
    same-engine nops inserted just before the over-subscribed one."""
    if getattr(tile.TileContext, "_drain_patched", False):
        return

    def _spill_excess_waits(nc):
        for fn in nc.m.functions:
            for bb in fn.blocks:
                insts = bb.instructions
                i = 0
                while i < len(insts):
                    inst = insts[i]
                    si = inst.sync_info
                    cap = 2 if isinstance(inst, mybir.InstEventSemaphore) else 1
                    if si is None or len(si.on_wait) <= cap:
                        i += 1
                        continue
                    extra = list(si.on_wait[cap:])
                    si.on_wait[:] = si.on_wait[:cap]
                    for w in extra:
                        nop = nc.engines[inst.engine].nop(
                            hint="wait_spill", nofuse=True
                        )
                        cur = nc.cur_bb.bb.instructions
                        cur.remove(nop.ins)
                        if nop.ins.sync_info is None:
                            nop.ins.sync_info = mybir.SyncInfo(
                                on_update=[], on_wait=[]
                            )
                        nop.ins.sync_info.on_wait.append(w)
                        insts.insert(i, nop.ins)
                        i += 1
                    i += 1

    def _drain_and_barrier(self, tick_clock, wait_clock):
        nc = self.nc
        drain_inst = nc.sync.drain()
        wait_clock.add_sem_waits(
            drain_inst.ins, ScopedClock({None: tick_clock.global_clock})
        )
        nc.all_engine_barrier()
        assert self.sems is not None
        popped = nc._tile_sem_poison_stack.pop()
        assert popped is self._sem_poison
        nc.clear_and_free_semaphores(list(self.sems.allocated().values()))
        nc.all_engine_barrier()
        _spill_excess_waits(nc)

    tile.TileContext._drain_and_barrier = _drain_and_barrier
    tile.TileContext._drain_patched = True


def _build_program():
    _patch_tail_drain()
    nc = bass.Bass()

    xt = nc.dram_tensor("xt", [128, NCJ, N], BF16, kind="ExternalInput")
    # host-prearranged weight layouts (see kernel() below)
    w_q = nc.dram_tensor("w_q", [128, NCJ, HPC, HD], BF16, kind="ExternalInput")
    w_k = nc.dram_tensor("w_k", [128, NCJ, HPC, HD], BF16, kind="ExternalInput")
    w_v = nc.dram_tensor("w_v", [128, NCJ, HPC * HD], BF16, kind="ExternalInput")
    w_o = nc.dram_tensor("w_o", [128, HPC, C], BF16, kind="ExternalInput")
    b_q = nc.dram_tensor("b_q", [HD, HPC], F32, kind="ExternalInput")
    b_k = nc.dram_tensor("b_k", [HD, HPC], F32, kind="ExternalInput")
    out = nc.dram_tensor("out", [N, C], F32, kind="ExternalOutput")
    scratch = nc.dram_tensor("scratch", [NUNITS, 512], F32)

    from contextlib import ExitStack

    with tile.TileContext(nc) as tc, ExitStack() as ctx:
        const = ctx.enter_context(tc.tile_pool(name="const", bufs=1))
        w_q_sb = const.tile([128, NCJ, HPC, HD], BF16)
        w_k_sb = const.tile([128, NCJ, HPC, HD], BF16)
        w_v_sb = const.tile([128, NCJ, HPC * HD], BF16)
        w_o_sb = const.tile([128, HPC, C], BF16)
        b_q_sb = const.tile([HD, HPC], F32)
        b_k_sb = const.tile([HD, HPC], F32)

        nc.gpsimd.dma_start(out=w_q_sb, in_=w_q[:])
        nc.gpsimd.dma_start(out=w_k_sb, in_=w_k[:])
        nc.gpsimd.dma_start(out=w_v_sb, in_=w_v[:])
        nc.gpsimd.dma_start(out=w_o_sb, in_=w_o[:])
        nc.gpsimd.dma_start(out=b_q_sb, in_=b_q[:])
        nc.gpsimd.dma_start(out=b_k_sb, in_=b_k[:])

        persist = ctx.enter_context(tc.tile_pool(name="persist", bufs=1))
        # K=128 zero-padded: rows 0:64 hold the head's q/k rows, 64:128 stay
        # zero (K=64 M=128 single-group matmuls run at half rate on this HW)
        qT = persist.tile([128, HPC, N], BF16)
        kT = persist.tile([128, HPC, N], BF16)
        nc.gpsimd.memset(qT[HD:128, :, :], 0.0)
        nc.gpsimd.memset(kT[HD:128, :, :], 0.0)
        # [tokens, kb, head, 128]: dims at 0:64, ones at 64 (from memset)
        v_nat = persist.tile([128, NT, HPC, 128], BF16)

        # ---- fused pipeline: qkv production interleaved into attention ----
        with (
            tc.tile_pool(name="xTp", bufs=1) as xTp,
            tc.tile_pool(name="oTp", bufs=1) as oTp,
            tc.tile_pool(name="expp", bufs=4) as expp,
            tc.tile_pool(name="recipp", bufs=2) as recipp,
            tc.tile_pool(name="bcsb", bufs=2) as bcsb,
            tc.tile_pool(name="ostage", bufs=3) as ostage,
            tc.tile_pool(name="ps_s", bufs=2, space="PSUM") as ps_s,
            tc.tile_pool(name="ps_o", bufs=2, space="PSUM") as ps_o,
            tc.tile_pool(name="ps_p", bufs=2, space="PSUM") as ps_p,
        ):
            # oT zero-padded to K=128 (rows 64:128 stay 0; w_o rows there are
            # host-zeroed) so the projection avoids the K=64/M=128 slow path
            oT = oTp.tile([128, HPC, N], BF16)
            nc.gpsimd.memset(oT[HD:128, :, :], 0.0)
            xT = xTp.tile([128, NCJ, N], BF16)
            for tb in range(QB):
                tsl = slice(tb * 512, (tb + 1) * 512)
                nc.sync.dma_start(out=xT[:, :, tsl], in_=xt[:, :, tsl])
            nc.vector.memset(v_nat, 1.0)

            def production(tb):
                """qkv projections for one 512-token block; psum via the
                shared 'pp' tag (temporally disjoint from proj use)."""
                tsl = slice(tb * 512, (tb + 1) * 512)
                for kb in range(tb * 4, tb * 4 + 4):
                    ksl = slice(kb * 128, (kb + 1) * 128)
                    pv_ = ps_p.tile([128, HPC * HD], F32, tag="pp", name="pv_")
                    for cj in range(NCJ):
                        nc.tensor.matmul(
                            pv_,
                            lhsT=xT[:, cj, ksl],
                            rhs=w_v_sb[:, cj, :],
                            start=(cj == 0),
                            stop=(cj == NCJ - 1),
                        )
                    nc.vector.tensor_copy(
                        out=v_nat[:, kb, :, 0:HD],
                        in_=pv_.rearrange("p (h d) -> p h d", h=HPC),
                    )
                for w_sb, b_sb, dst in (
                    (w_k_sb, b_k_sb, kT),
                    (w_q_sb, b_q_sb, qT),
                ):
                    for h in range(HPC):
                        pm = ps_p.tile([HD, 512], F32, tag="pp", name="pm")
                        for cj in range(NCJ):
                            nc.tensor.matmul(
                                pm,
                                lhsT=w_sb[:, cj, h, :],
                                rhs=xT[:, cj, tsl],
                                start=(cj == 0),
                                stop=(cj == NCJ - 1),
                            )
                        nc.vector.tensor_scalar_add(
                            out=dst[0:HD, h, tsl], in0=pm,
                            scalar1=b_sb[:, h:h + 1],
                        )

            production(0)
            last_exp = {"inst": None}
            pending_recip = []  # flushed @g1 of the following unit (DVE)
            pending_bc = []     # flushed @g5 (PE bcast + DVE normalize)
            pending_proj = []   # flushed @g7 (PE matmuls)

            def make_tail(po, h, qsl, u):
                state = {}

                def recip():
                    rt = recipp.tile([1, 512], F32, name="rt")
                    nc.vector.reciprocal(out=rt, in_=po[HD:HD + 1, :])
                    # round-trip through DRAM to broadcast across partitions
                    # (no on-chip partition-broadcast path); explicit dep
                    # edge orders the read-back after the write
                    state["wr"] = nc.sync.dma_start(
                        out=scratch[u:u + 1, :], in_=rt
                    )

                def bcmult():
                    bc = bcsb.tile([HD, 512], F32, name="bc")
                    rd = nc.gpsimd.dma_start(
                        out=bc,
                        in_=bass.AP(
                            tensor=scratch, offset=u * 512,
                            ap=[[0, HD], [1, 512]],
                        ),
                    )
                    add_dep_helper(
                        rd.ins, state["wr"].ins, sync=True,
                        reason="scratch broadcast RAW",
                    )
                    nc.vector.tensor_mul(
                        out=oT[0:HD, h, qsl], in0=po[0:HD, :], in1=bc
                    )
                return recip, bcmult

            def make_proj(qb, j):
                def proj():
                    q0 = qb * 512 + j * 128
                    pp = ps_p.tile([128, C], F32, name="pp")
                    for h in range(HPC):
                        mm = nc.tensor.matmul(
                            pp,
                            lhsT=oT[:, h, q0:q0 + 128],
                            rhs=w_o_sb[:, h, :],
                            start=(h == 0),
                            stop=(h == HPC - 1),
                        )
                        if h == 0 and last_exp["inst"] is not None:
                            add_dep_helper(
                                mm.ins, last_exp["inst"], sync=False,
                                reason="proj after normalize really done",
                            )
                    ot = ostage.tile([128, C], F32, name="ot")
                    nc.vector.tensor_copy(out=ot, in_=pp)
                    nc.sync.dma_start(out=out[q0:q0 + 128, :], in_=ot)
                return proj

            units = [(qb, h) for qb in range(QB) for h in range(HPC)]

            def s_group(u, g):
                qb, h = units[u]
                qsl = slice(qb * 512, (qb + 1) * 512)
                ks = ps_s.tile([128, GP, 512], F32, name="ks")
                for j in range(GP):
                    kb = g * GP + j
                    nc.tensor.matmul(
                        ks[:, j, :],
                        lhsT=kT[:, h, kb * 128:(kb + 1) * 128],
                        rhs=qT[:, h, qsl],
                        start=True,
                        stop=True,
                    )
                return ks

            # flat (unit, group) pipeline: the scores skew carries across
            # unit boundaries so the PE/ACT streams never drain
            flat = [(u, g) for u in range(len(units)) for g in range(NG)]
            po_tiles = {}
            pend = s_group(*flat[0])
            for i, (u, g) in enumerate(flat):
                ks = pend
                pend = s_group(*flat[i + 1]) if i + 1 < len(flat) else None
                if u == 0 and g % 2 == 0 and g // 2 + 1 < QB:
                    production(g // 2 + 1)
                if g == 1:
                    for f in pending_recip:
                        f()
                    pending_recip.clear()
                elif g == 8:
                    for f in pending_bc:
                        f()
                    pending_bc.clear()
                elif g in (9, 11, 13, 15) and pending_proj:
                    pending_proj.pop(0)()
                et = expp.tile([128, GP, 512], BF16)
                exp_bi = nc.scalar.activation(
                    out=et, in_=ks, func=AF.Exp, scale=SCALE
                )
                last_exp["inst"] = exp_bi.ins
                qb, h = units[u]
                if g == 0:
                    po_tiles[u] = ps_o.tile([HD + 1, 512], F32, name="po")
                po = po_tiles[u]
                for j in range(GP):
                    kb = g * GP + j
                    nc.tensor.matmul(
                        po,
                        lhsT=v_nat[:, kb, h, 0:HD + 1],
                        rhs=et[:, j, :],
                        start=(kb == 0),
                        stop=(kb == NT - 1),
                    )
                if g == NG - 1:
                    qsl = slice(qb * 512, (qb + 1) * 512)
                    recip, bcmult = make_tail(po_tiles.pop(u), h, qsl, u)
                    pending_recip.append(recip)
                    pending_bc.append(bcmult)
                    if h == HPC - 1:
                        for j in range(4):
                            pending_proj.append(make_proj(qb, j))
            for f in pending_recip:
                f()
            for f in pending_bc:
                f()
            for f in pending_proj:
                f()

    return nc


_PROGRAM = None


def _get_program():
    global _PROGRAM
    if _PROGRAM is None:
        _PROGRAM = _build_program()
    return _PROGRAM


def _bf16(a):
    import ml_dtypes

    return np.asarray(a, dtype=np.float32).astype(ml_dtypes.bfloat16)


def _prep_core_inputs(x, W_qkv, b_qkv, heads, batch):
    """Host-side slicing/relayout for one core."""
    cols = np.concatenate([np.arange(h * HD, (h + 1) * HD) for h in heads])
    w_q = W_qkv[:, cols]               # [512, 128]
    w_k = W_qkv[:, C + cols]
    w_v = W_qkv[:, 2 * C + cols]
    # [512, 128] -> [128 partitions, NCJ chunks, ...]
    w_q = np.ascontiguousarray(
        w_q.reshape(NCJ, 128, HPC, HD).transpose(1, 0, 2, 3))
    w_k = np.ascontiguousarray(
        w_k.reshape(NCJ, 128, HPC, HD).transpose(1, 0, 2, 3))
    w_v = np.ascontiguousarray(
        w_v.reshape(NCJ, 128, HPC * HD).transpose(1, 0, 2))
    b_q = np.ascontiguousarray(b_qkv[cols].reshape(HPC, HD).T)      # [64, 2]
    b_k = np.ascontiguousarray(b_qkv[C + cols].reshape(HPC, HD).T)
    xt = np.ascontiguousarray(
        x[batch].T.reshape(NCJ, 128, N).transpose(1, 0, 2))
    return {
        "xt": _bf16(xt),
        "w_q": _bf16(w_q),
        "w_k": _bf16(w_k),
        "w_v": _bf16(w_v),
        "b_q": b_q.astype(np.float32),
        "b_k": b_k.astype(np.float32),
    }


def _core_w_o(W_out, heads):
    rows = np.concatenate([np.arange(h * HD, (h + 1) * HD) for h in heads])
    w = np.zeros((128, HPC, C), dtype=np.float32)
    w[0:HD] = W_out[rows].reshape(HPC, HD, C).transpose(1, 0, 2)
    return _bf16(w)


def kernel(x, W_qkv, b_qkv, W_out, b_out):
    x = np.asarray(x, dtype=np.float32)
    W_qkv = np.asarray(W_qkv, dtype=np.float32)
    b_qkv = np.asarray(b_qkv, dtype=np.float32)
    W_out = np.asarray(W_out, dtype=np.float32)
    b_out = np.asarray(b_out, dtype=np.float32)

    nc = _get_program()
    in_maps = []
    for c in range(NCORES):
        batch, hp = c // 4, c % 4
        heads = [2 * hp, 2 * hp + 1]
        im = _prep_core_inputs(x, W_qkv, b_qkv, heads, batch)
        im["w_o"] = _core_w_o(W_out, heads)
        in_maps.append(im)

    res = run_bass_kernel_spmd(nc, in_maps, core_ids=list(range(NCORES)))

    # v-bias commutes: softmax rows sum to 1, so (P @ (V + 1 b_v)) @ W_o
    # = P@V@W_o + b_v@W_o. Add b_v@W_out and b_out once on the host.
    const_row = b_qkv[2 * C:] @ W_out + b_out    # [512]
    out = np.empty((B, N, C), dtype=np.float32)
    for b in range(B):
        acc = res.results[4 * b]["out"].astype(np.float32).copy()
        for c in range(4 * b + 1, 4 * b + 4):
            acc += res.results[c]["out"]
        out[b] = acc + const_row
    return out
